# revision 1
# baseline (speedup 1.0000x reference)
"""Trainium2 Bass kernel for an MQA attention block (8 q-heads, shared K/V).

Sharding: 8 cores; core c -> batch b=c//4, query rows s0=(c%4)*512 .. +512,
all 8 heads.  K/V (full sequence, per batch) are computed redundantly on each
core; there is no cross-core communication.  Host folds the two RMSBatchNorm
evals into the projection weights, and DQ^-0.5 into the q-layernorm affine.

Projections run in feature-on-partitions ("T") layout so every matmul keeps a
512-wide moving dim (full-rate float32r).  LayerNorm + RoPE run in row layout
(positions on partitions) after a PE transpose, so LN stats are per-partition
scalars and per-feature affines are host-replicated constant tiles.  Softmax
needs no max-subtraction (logits softcapped to +-5); the denominator comes
from a ones-column appended to V.
"""

import os
import sys

for _p in ("/opt/trn_rl_repo",):
    if _p not in sys.path and os.path.isdir(_p):
        sys.path.insert(0, _p)

import numpy as np
from contextlib import ExitStack

import concourse.bass as bass
import concourse.mybir as mybir
import concourse.tile as tile
from concourse import bacc
from concourse import bass_utils

F32 = mybir.dt.float32
F32R = mybir.dt.float32r

# problem shapes (hardcoded per contract)
B, S, D = 2, 2048, 1536
H, DQ, DK, DV = 8, 128, 128, 192
P = 128
SQ = S // 4          # 512 query rows per core
DC = D // P          # 12 contraction chunks
JC = S // P          # 16 key chunks
SC = SQ // P         # 4 query-row chunks
NCORES = 8
EPS_RMS = 1e-6
EPS_LN = 1e-5
SOFTCAP = 5.0
ROPE_BASE = 8192.0
VPAD = 256           # v' row width: 192 v + 1 ones + 63 zero (N>=256 for f32r)
HALF = DQ // 2


def _r(ap):
    """bitcast an fp32 AP to float32r for full-rate PE matmuls"""
    return ap.bitcast(F32R)


PHASE_LIMIT = {"kv": 1, "q": 2, "attn": 3, "full": 4}[
    os.environ.get("KERNEL_PHASES", "full")
]
REPEAT = int(os.environ.get("KERNEL_REPEAT", "1"))


def build_program(repeat=None):
    global REPEAT
    if repeat is not None:
        REPEAT = repeat
    nc = bacc.Bacc(
        "TRN2", target_bir_lowering=False, debug=False, num_devices=NCORES
    )

    def din(name, shape):
        return nc.dram_tensor(name, list(shape), F32, kind="ExternalInput").ap()

    # per-core inputs
    xT = din("xT", (D, S))
    xTq = din("xTq", (D, SQ))
    biasT = din("biasT", (S, SQ))
    cosq_t = din("cosq", (SQ, HALF))
    sinq_t = din("sinq", (SQ, HALF))
    # shared (replicated) inputs
    cosk_t = din("cosk", (S, HALF))
    sink_t = din("sink", (S, HALF))
    wq = din("wq", (D, H * DQ))
    wk = din("wk", (D, DK))
    wv = din("wv", (D, DV))
    wo = din("wo", (H * DV, D))
    bq_b = din("bq", (P, H))      # folded rms1 bias through Wq, per (d, h)
    bk_b = din("bk", (P, 1))
    bv_b = din("bv", (DV, 1))
    qgr_t = din("qgr", (P, DQ))   # row-replicated LN affines
    qbr_t = din("qbr", (P, DQ))
    kgr_t = din("kgr", (P, DK))
    kbr_t = din("kbr", (P, DK))
    vgr_t = din("vgr", (P, DV))
    vbr_t = din("vbr", (P, DV))
    bor_t = din("bor", (P, D))    # row-replicated output bias
    vpad_t = din("vpad", (P, JC * (VPAD - DV)))  # ones-col + zero pad for v'
    ident = din("ident", (P, P))
    out = nc.dram_tensor("out", [SQ, D], F32, kind="ExternalOutput").ap()

    TT = mybir.AluOpType
    AF = mybir.ActivationFunctionType
    AX = mybir.AxisListType

    with tile.TileContext(nc) as tc, ExitStack() as ctx:
        const = ctx.enter_context(tc.tile_pool(name="const", bufs=1))
        persist = ctx.enter_context(tc.tile_pool(name="persist", bufs=1))

        # ---- small constants into SBUF (live whole kernel)
        ident_sb = const.tile([P, P], F32)
        nc.sync.dma_start(ident_sb[:], ident)
        bk_sb = const.tile([P, 1], F32)
        nc.sync.dma_start(bk_sb[:], bk_b)
        bvA = const.tile([P, 1], F32)
        nc.sync.dma_start(bvA[:], bv_b[:P, :])
        bvB = const.tile([DV - P, 1], F32)
        nc.sync.dma_start(bvB[:], bv_b[P:, :])
        bqh_sb = const.tile([P, H], F32)
        nc.sync.dma_start(bqh_sb[:], bq_b)
        qgr = const.tile([P, DQ], F32)
        nc.sync.dma_start(qgr[:], qgr_t)
        qbr = const.tile([P, DQ], F32)
        nc.sync.dma_start(qbr[:], qbr_t)
        kgr = const.tile([P, DK], F32)
        nc.sync.dma_start(kgr[:], kgr_t)
        kbr = const.tile([P, DK], F32)
        nc.sync.dma_start(kbr[:], kbr_t)
        vgr = const.tile([P, DV], F32)
        nc.sync.dma_start(vgr[:], vgr_t)
        vbr = const.tile([P, DV], F32)
        nc.sync.dma_start(vbr[:], vbr_t)
        bor = const.tile([P, D], F32)
        nc.sync.dma_start(bor[:], bor_t)
        eps_sb = const.tile([P, 1], F32)
        nc.vector.memset(eps_sb[:], EPS_LN)

        # persistent activation tensors
        kT_sb = persist.tile([P, S], F32)            # rope'd LN'd k, [dk, s]
        vrow_sb = persist.tile([P, JC, VPAD], F32)   # v rows + ones col
        qT_sb = persist.tile([P, H, SQ], F32)        # rope'd LN'd q, [dq,h,i]
        yatt_sb = persist.tile([P, SC, H * DV], F32)  # attn out rows

        nc.sync.dma_start(
            _r(vrow_sb[:, :, DV:]),
            _r(vpad_t.rearrange("p (jc f) -> p jc f", jc=JC)),
        )

        def ln_rows(pool, src_sb, width, inv_n, grep, brep, out_ap, tag):
            """LayerNorm rows of src_sb [P, width] over the free dim, then
            out = norm * grep + brep, written to out_ap ([P, width])."""
            st = pool.tile([P, 4], F32, tag=tag + "st")
            nc.vector.tensor_reduce(
                st[:, 0:1], src_sb[:], axis=AX.X, op=TT.add
            )
            sq = pool.tile([P, width], F32, tag=tag + "sq")
            nc.vector.tensor_tensor(sq[:], src_sb[:], src_sb[:], TT.mult)
            nc.vector.tensor_reduce(
                st[:, 1:2], sq[:], axis=AX.X, op=TT.add
            )
            # [mu, m2] = [sum, sumsq] * inv_n
            nc.vector.tensor_scalar(
                st[:, 0:2], st[:, 0:2], inv_n, None, TT.mult
            )
            nc.vector.tensor_tensor(st[:, 2:3], st[:, 0:1], st[:, 0:1],
                                    TT.mult)
            nc.vector.tensor_tensor(st[:, 3:4], st[:, 1:2], st[:, 2:3],
                                    TT.subtract)
            nc.scalar.activation(
                st[:, 3:4], st[:, 3:4], AF.Sqrt, bias=eps_sb[:, 0:1]
            )
            nc.vector.reciprocal(st[:, 3:4], st[:, 3:4])
            xn = pool.tile([P, width], F32, tag=tag + "xn")
            nc.vector.tensor_scalar(
                xn[:], src_sb[:], st[:, 0:1], st[:, 3:4],
                TT.subtract, TT.mult,
            )
            nc.vector.tensor_tensor(xn[:], xn[:], grep[:], TT.mult)
            nc.vector.tensor_tensor(out_ap, xn[:], brep[:], TT.add)
            return out_ap

        def rope_rows(pool, xn, cos_t, sin_t, out_ap, tag):
            """out[:, :64] = x1*cos - x2*sin ; out[:, 64:] = x1*sin + x2*cos"""
            x1 = xn[:, :HALF]
            x2 = xn[:, HALF:]
            t1 = pool.tile([P, HALF], F32, tag=tag + "t1")
            nc.vector.tensor_tensor(out_ap[:, :HALF], x1, cos_t, TT.mult)
            nc.vector.tensor_tensor(t1[:], x2, sin_t, TT.mult)
            nc.vector.tensor_tensor(
                out_ap[:, :HALF], out_ap[:, :HALF], t1[:], TT.subtract
            )
            nc.vector.tensor_tensor(out_ap[:, HALF:], x1, sin_t, TT.mult)
            nc.vector.tensor_tensor(t1[:], x2, cos_t, TT.mult)
            nc.vector.tensor_tensor(
                out_ap[:, HALF:], out_ap[:, HALF:], t1[:], TT.add
            )

        JH = S // 2  # columns per half

        for _rep in range(REPEAT):
          # ========================================================
          # Phase KV: k/v projections (T layout) + LN/rope (rows)
          # ========================================================

         with (
             tc.tile_pool(name="kvc", bufs=1) as kvc,
             tc.tile_pool(name="kvbig", bufs=1) as kvbig,
             tc.tile_pool(name="kvp", bufs=2) as kvp,
             tc.tile_pool(name="kvpsum", bufs=1, space="PSUM") as kvps,
         ):
             wk_sb = kvc.tile([P, DC, DK], F32)
             nc.sync.dma_start(_r(wk_sb[:]), _r(wk.rearrange("(c p) f -> p c f", p=P)))
             wv_sb = kvc.tile([P, DC, DV], F32)
             nc.sync.dma_start(_r(wv_sb[:]), _r(wv.rearrange("(c p) f -> p c f", p=P)))
             cosk_sb = kvc.tile([P, JC, HALF], F32)
             nc.sync.dma_start(
                 cosk_sb[:], cosk_t.rearrange("(jc p) f -> p jc f", p=P)
             )
             sink_sb = kvc.tile([P, JC, HALF], F32)
             nc.sync.dma_start(
                 sink_sb[:], sink_t.rearrange("(jc p) f -> p jc f", p=P)
             )

             for jh in range(2):
                 j0 = jh * JH
                 kT_ps = kvps.tile([P, JH], F32, tag="kT_ps")
                 vTa_ps = kvps.tile([P, JH], F32, tag="vTa_ps")
                 vTb_ps = kvps.tile([DV - P, JH], F32, tag="vTb_ps")
                 for dc in range(DC):
                     xt = kvp.tile([P, JH], F32, tag="xt")
                     nc.sync.dma_start(
                         _r(xt[:]), _r(xT[dc * P : (dc + 1) * P, j0 : j0 + JH])
                     )
                     for n in range(JH // 512):
                         cs = slice(n * 512, (n + 1) * 512)
                         nc.tensor.matmul(
                             kT_ps[:, cs],
                             _r(wk_sb[:, dc, :]),
                             _r(xt[:, cs]),
                             start=(dc == 0),
                             stop=(dc == DC - 1),
                         )
                         nc.tensor.matmul(
                             vTa_ps[:, cs],
                             _r(wv_sb[:, dc, :P]),
                             _r(xt[:, cs]),
                             start=(dc == 0),
                             stop=(dc == DC - 1),
                         )
                         nc.tensor.matmul(
                             vTb_ps[:, cs],
                             _r(wv_sb[:, dc, P:]),
                             _r(xt[:, cs]),
                             start=(dc == 0),
                             stop=(dc == DC - 1),
                         )

                 # drain to SBUF with folded rms1 bias (per-partition in T)
                 k_sb = kvbig.tile([P, JH], F32, tag="k_sb")
                 nc.vector.tensor_scalar_add(k_sb[:], kT_ps[:], bk_sb[:, 0:1])
                 va_sb = kvbig.tile([P, JH], F32, tag="va_sb")
                 nc.vector.tensor_scalar_add(va_sb[:], vTa_ps[:], bvA[:, 0:1])
                 vb_sb2 = kvbig.tile([DV - P, JH], F32, tag="vb_sb2")
                 nc.vector.tensor_scalar_add(
                     vb_sb2[:], vTb_ps[:], bvB[:, 0:1]
                 )

                 for t in range(8):
                     jc = jh * 8 + t
                     tsl = slice(t * P, (t + 1) * P)
                     # ---- k chunk: transpose -> rows
                     scr = kvps.tile([P, 512], F32, tag="scratch", name="scrk")
                     nc.tensor.transpose(scr[:, :P], k_sb[:, tsl], ident_sb[:])
                     krow = kvp.tile([P, P], F32, tag="krow")
                     nc.vector.tensor_copy(krow[:], scr[:, :P])
                     kn = kvp.tile([P, P], F32, tag="kn")
                     ln_rows(kvp, krow, DK, 1.0 / DK, kgr, kbr, kn[:], "k")
                     kr = kvp.tile([P, P], F32, tag="kr")
                     rope_rows(
                         kvp, kn, cosk_sb[:, jc, :], sink_sb[:, jc, :],
                         kr, "k",
                     )
                     # transpose back -> kT
                     scr2 = kvps.tile([P, 512], F32, tag="scratch",
                                      name="scrk2")
                     nc.tensor.transpose(scr2[:, :P], kr[:], ident_sb[:])
                     nc.vector.tensor_copy(
                         _r(kT_sb[:, jc * P : (jc + 1) * P]), scr2[:, :P]
                     )

                     # ---- v chunk: transpose a|b into one row tile
                     scr3 = kvps.tile([P, 512], F32, tag="scratch",
                                      name="scrv")
                     nc.tensor.transpose(
                         scr3[:, :P], va_sb[:, tsl], ident_sb[:]
                     )
                     nc.tensor.transpose(
                         scr3[:, P:DV], vb_sb2[:, tsl],
                         ident_sb[: DV - P, : DV - P],
                     )
                     vrow = kvp.tile([P, DV], F32, tag="vrow")
                     nc.vector.tensor_copy(vrow[:], scr3[:, :DV])
                     ln_rows(
                         kvp, vrow, DV, 1.0 / DV, vgr, vbr,
                         _r(vrow_sb[:, jc, :DV]), "v",
                     )

         # =========================================================
         # Phase Q: per-head q projection (T) + LN/rope (rows)
         # =========================================================
         if PHASE_LIMIT >= 2:
          with (
             tc.tile_pool(name="qc", bufs=1) as qc,
             tc.tile_pool(name="qw", bufs=2) as qw,
             tc.tile_pool(name="qp", bufs=2) as qp,
             tc.tile_pool(name="qpsum", bufs=2, space="PSUM") as qps,
         ):
             xtq_sb = qc.tile([P, DC, SQ], F32)
             nc.sync.dma_start(
                 _r(xtq_sb[:]), _r(xTq.rearrange("(c p) s -> p c s", p=P))
             )
             cosq_sb = qc.tile([P, SC, HALF], F32)
             nc.sync.dma_start(
                 cosq_sb[:], cosq_t.rearrange("(sc p) f -> p sc f", p=P)
             )
             sinq_sb = qc.tile([P, SC, HALF], F32)
             nc.sync.dma_start(
                 sinq_sb[:], sinq_t.rearrange("(sc p) f -> p sc f", p=P)
             )

             wq3 = wq.rearrange("(c p) f -> p c f", p=P)
             for h in range(H):
                 wqh = qw.tile([P, DC, DQ], F32, tag="wqh")
                 nc.sync.dma_start(
                     _r(wqh[:]), _r(wq3[:, :, h * DQ : (h + 1) * DQ])
                 )
                 q_ps = qps.tile([P, SQ], F32, tag="q_ps")
                 for dc in range(DC):
                     nc.tensor.matmul(
                         q_ps[:],
                         _r(wqh[:, dc, :]),
                         _r(xtq_sb[:, dc, :]),
                         start=(dc == 0),
                         stop=(dc == DC - 1),
                     )
                 q_sb = qp.tile([P, SQ], F32, tag="q_sb")
                 nc.vector.tensor_scalar_add(
                     q_sb[:], q_ps[:], bqh_sb[:, h : h + 1]
                 )
                 for sc in range(SC):
                     ssl = slice(sc * P, (sc + 1) * P)
                     scr = qps.tile([P, 512], F32, tag="qscr", name="qscr")
                     nc.tensor.transpose(
                         scr[:, :P], q_sb[:, ssl], ident_sb[:]
                     )
                     qrow = qp.tile([P, P], F32, tag="qrow")
                     nc.vector.tensor_copy(qrow[:], scr[:, :P])
                     qn = qp.tile([P, P], F32, tag="qn")
                     ln_rows(qp, qrow, DQ, 1.0 / DQ, qgr, qbr, qn[:], "q")
                     qr = qp.tile([P, P], F32, tag="qr")
                     rope_rows(
                         qp, qn, cosq_sb[:, sc, :], sinq_sb[:, sc, :],
                         qr, "q",
                     )
                     scr2 = qps.tile([P, 512], F32, tag="qscr", name="qscr2")
                     nc.tensor.transpose(scr2[:, :P], qr[:], ident_sb[:])
                     nc.vector.tensor_copy(
                         _r(qT_sb[:, h, sc * P : (sc + 1) * P]), scr2[:, :P]
                     )

         # =========================================================
         # Attention phase (per head, groups of 2 key chunks)
         # =========================================================
         if PHASE_LIMIT >= 3:
          with tc.tile_pool(name="wop", bufs=1) as wop:
             # prefetch wo during attention
             wo_sb = wop.tile([P, DC, D], F32)
             nc.sync.dma_start(_r(wo_sb[:]), _r(wo.rearrange("(c p) f -> p c f", p=P)))

             with (
                 tc.tile_pool(name="att", bufs=3) as att,
                 tc.tile_pool(name="attb", bufs=1) as attb,
                 tc.tile_pool(name="apsum", bufs=1, space="PSUM") as aps,
                 tc.tile_pool(name="ypsum", bufs=1, space="PSUM") as yps,
             ):
                 biasT_sb = attb.tile([P, JC, SQ], F32)
                 nc.sync.dma_start(
                     biasT_sb[:], biasT.rearrange("(jc p) i -> p jc i", p=P)
                 )

                 y_ps = [
                     yps.tile([P, VPAD], F32, tag=f"y{ic}", name=f"y{ic}")
                     for ic in range(SC)
                 ]
                 for h in range(H):
                     for jg in range(JC // 2):
                         tag = "pq" + str(jg % 2)
                         pq = aps.tile([P, 2, 512], F32, tag=tag, name="pq")
                         for c in range(2):
                             jc = jg * 2 + c
                             nc.tensor.matmul(
                                 pq[:, c, :],
                                 _r(kT_sb[:, jc * P : (jc + 1) * P]),
                                 _r(qT_sb[:, h, :]),
                                 start=True, stop=True,
                             )
                         nc.vector.tensor_tensor(
                             pq[:], pq[:],
                             biasT_sb[:, jg * 2 : jg * 2 + 2, :], TT.add,
                         )
                         nc.scalar.activation(
                             pq[:], pq[:], AF.Tanh, scale=1.0 / SOFTCAP
                         )
                         pt = att.tile([P, 2, 512], F32, tag="pt")
                         nc.scalar.activation(
                             _r(pt[:]), pq[:], AF.Exp, scale=SOFTCAP
                         )
                         for c in range(2):
                             jc = jg * 2 + c
                             for ic in range(SC):
                                 nc.tensor.matmul(
                                     y_ps[ic][:],
                                     _r(pt[:, c, ic * P : (ic + 1) * P]),
                                     _r(vrow_sb[:, jc, :]),
                                     start=(jc == 0),
                                     stop=(jc == JC - 1),
                                 )
                     # drain: normalize rows by the ones-column sum
                     for ic in range(SC):
                         recip = att.tile([P, 1], F32, tag="recip")
                         nc.vector.reciprocal(
                             recip[:], y_ps[ic][:, DV : DV + 1]
                         )
                         nc.vector.tensor_scalar(
                             yatt_sb[:, ic, h * DV : (h + 1) * DV],
                             y_ps[ic][:, :DV],
                             recip[:, 0:1], None, TT.mult,
                         )

             # =====================================================
             # Output projection: transpose y_att, then matmul + bias
             # =====================================================
             if PHASE_LIMIT >= 4:
              with (
                 tc.tile_pool(name="op", bufs=2) as op,
                 tc.tile_pool(name="oyT", bufs=1) as oyT,
                 tc.tile_pool(name="opsum", bufs=2, space="PSUM") as ops,
                 tc.tile_pool(name="otps", bufs=2, space="PSUM") as otps,
             ):
                 yT_sb = oyT.tile([P, DC, SQ], F32, tag="yT")
                 for sc in range(SC):
                     for fc in range(DC):
                         pt2 = otps.tile([P, P], F32, tag="yt")
                         nc.tensor.transpose(
                             pt2[:],
                             yatt_sb[:, sc, fc * P : (fc + 1) * P],
                             ident_sb[:],
                         )
                         nc.vector.tensor_copy(
                             _r(yT_sb[:, fc, sc * P : (sc + 1) * P]), pt2[:]
                         )
                 for sc in range(SC):
                     o_ps = ops.tile([P, D], F32, tag="o_ps")
                     for fc in range(DC):
                         for n in range(D // 512):
                             nc.tensor.matmul(
                                 o_ps[:, n * 512 : (n + 1) * 512],
                                 _r(yT_sb[:, fc, sc * P : (sc + 1) * P]),
                                 _r(wo_sb[:, fc, n * 512 : (n + 1) * 512]),
                                 start=(fc == 0),
                                 stop=(fc == DC - 1),
                             )
                     o_sb = op.tile([P, D], F32, tag="o_sb")
                     nc.vector.tensor_tensor(
                         o_sb[:], o_ps[:], bor[:], TT.add
                     )
                     nc.sync.dma_start(
                         out[sc * P : (sc + 1) * P, :], o_sb[:]
                     )

         if PHASE_LIMIT < 4:
             _finish_debug(nc, tc, out)

    nc.compile()
    return nc


def _finish_debug(nc, tc, out):
    F = mybir.dt.float32
    with tc.tile_pool(name="dbg", bufs=1) as dbg:
        z = dbg.tile([P, D], F)
        nc.vector.memset(z[:], 0.0)
        for sc in range(SC):
            nc.sync.dma_start(out[sc * P : (sc + 1) * P, :], z[:])


def _host_prep(inputs):
    f32 = np.float32
    x = np.asarray(inputs["x"], f32)
    bias = np.asarray(inputs["attention_bias"], f32)
    g1 = np.asarray(inputs["g1"], f32)
    b1 = np.asarray(inputs["b1"], f32)
    rr1 = np.asarray(inputs["rrms1"], f32)
    Wq = np.asarray(inputs["Wq"], f32)
    Wk = np.asarray(inputs["Wk"], f32)
    Wv = np.asarray(inputs["Wv"], f32)
    qg = np.asarray(inputs["qg"], f32)
    qb = np.asarray(inputs["qb"], f32)
    kg = np.asarray(inputs["kg"], f32)
    kb = np.asarray(inputs["kb"], f32)
    vg = np.asarray(inputs["vg"], f32)
    vb = np.asarray(inputs["vb"], f32)
    Wo = np.asarray(inputs["Wo"], f32)
    bo = np.asarray(inputs["bo"], f32)
    g2 = np.asarray(inputs["g2"], f32)
    b2 = np.asarray(inputs["b2"], f32)
    rr2 = np.asarray(inputs["rrms2"], f32)

    scale1 = (g1 * (1.0 / np.sqrt(rr1 + EPS_RMS))).astype(f32)
    Wq_e = (Wq * scale1[:, None]).astype(f32)
    Wk_e = (Wk * scale1[:, None]).astype(f32)
    Wv_e = (Wv * scale1[:, None]).astype(f32)
    bq_row = (b1 @ Wq).astype(f32)      # [H*DQ]
    bk_row = (b1 @ Wk).astype(f32)      # [DK]
    bv_row = (b1 @ Wv).astype(f32)      # [DV]
    sc_q = f32(DQ) ** f32(-0.5)
    qg_e = (qg * sc_q).astype(f32)
    qb_e = (qb * sc_q).astype(f32)
    scale2 = (g2 * (1.0 / np.sqrt(rr2 + EPS_RMS))).astype(f32)
    Wo_e = (Wo * scale2[None, :]).astype(f32)
    bo_e = (bo * scale2 + b2).astype(f32)

    freqs = (
        1.0 / (ROPE_BASE ** (np.arange(HALF, dtype=f32) / HALF))
    ).astype(f32)
    ang = np.arange(S, dtype=f32)[:, None] * freqs[None, :]
    cos = np.cos(ang).astype(f32)                        # [S, 64]
    sin = np.sin(ang).astype(f32)

    rep = lambda v: np.broadcast_to(v[None, :], (P, v.shape[0]))
    shared = {
        "cosk": cos,
        "sink": sin,
        "wq": Wq_e,
        "wk": Wk_e,
        "wv": Wv_e,
        "wo": Wo_e,
        "bq": bq_row.reshape(H, DQ).T,
        "bk": bk_row.reshape(DK, 1),
        "bv": bv_row.reshape(DV, 1),
        "qgr": rep(qg_e),
        "qbr": rep(qb_e),
        "kgr": rep(kg),
        "kbr": rep(kb),
        "vgr": rep(vg),
        "vbr": rep(vb),
        "bor": rep(bo_e),
        "vpad": np.tile(
            np.concatenate(
                [np.ones((P, 1), f32), np.zeros((P, VPAD - DV - 1), f32)],
                axis=1,
            ),
            (1, JC),
        ),
        "ident": np.eye(P, dtype=f32),
    }
    shared = {k: np.ascontiguousarray(v, dtype=f32) for k, v in shared.items()}

    xTs = [np.ascontiguousarray(x[b].T) for b in range(B)]
    in_maps = []
    for c in range(NCORES):
        b = c // 4
        s0 = (c % 4) * SQ
        m = dict(shared)
        m["xT"] = xTs[b]
        m["xTq"] = np.ascontiguousarray(xTs[b][:, s0 : s0 + SQ])
        m["biasT"] = np.ascontiguousarray(bias[0, 0, s0 : s0 + SQ, :].T)
        m["cosq"] = np.ascontiguousarray(cos[s0 : s0 + SQ, :])
        m["sinq"] = np.ascontiguousarray(sin[s0 : s0 + SQ, :])
        in_maps.append(m)
    return in_maps


_NC_CACHE = None


def _get_nc():
    global _NC_CACHE
    if _NC_CACHE is None:
        _NC_CACHE = build_program()
    return _NC_CACHE


def kernel(**inputs) -> np.ndarray:
    nc = _get_nc()
    in_maps = _host_prep(inputs)
    res = bass_utils.run_bass_kernel_spmd(
        nc, in_maps, core_ids=list(range(NCORES))
    )
    outs = res.results
    full = np.empty((B, S, D), np.float32)
    for c in range(NCORES):
        b = c // 4
        s0 = (c % 4) * SQ
        full[b, s0 : s0 + SQ, :] = outs[c]["out"]
    return full


if __name__ == "__main__":
    nc = _get_nc()
    print("build + compile OK")



# revision 12
# speedup vs baseline: 1.5004x; 1.5004x over previous
"""Trainium2 Bass kernel for an MQA attention block (8 q-heads, shared K/V).

Sharding: 8 cores; core c -> batch b=c//4, query rows s0=(c%4)*512 .. +512,
all 8 heads.  K/V (full sequence, per batch) are computed redundantly on each
core; no cross-core communication.

Layout strategy ("row" architecture):
 - K/V projection computed jointly in ROW layout (keys on partitions): per
   key-chunk, lhsT = xT chunk (stationary), rhs = [Wk|Wv] (moving, N=320).
   LayerNorm stats are per-partition; affines are folded into rope tables
   (k) and into Wo/bo (v).  v rows written bf16, consumed directly by the
   attention y-matmul; k rows rope'd then PE-transposed to kT.
 - Q projection in ROW layout per half-head-group (N=512), LN+rope in rows
   (affine + DQ^-0.5 folded into tables), PE-transposed to qT.
 - Attention: logits = kT.T @ qT in fp32r; softcap chain is one DVE
   scalar_tensor_tensor (x0.2 + bias, writes bf16) + ACT tanh + ACT exp
   (both bf16, 2x rate); y accumulates p.T @ [v|1|pad] (bf16, N=200) with
   the ones-column giving the softmax denominator.
 - Output projection: yatt rows (bf16) -> PE transpose (bf16) -> yT,
   matmul with bf16 Wo (v-affine + rms2 folded), add bias, DMA out.
"""

import os
import sys

for _p in ("/opt/trn_rl_repo",):
    if _p not in sys.path and os.path.isdir(_p):
        sys.path.insert(0, _p)

import numpy as np
from contextlib import ExitStack

import concourse.bass as bass
import concourse.mybir as mybir
import concourse.tile as tile
from concourse import bacc
from concourse import bass_utils

F32 = mybir.dt.float32
F32R = mybir.dt.float32r
BF16 = mybir.dt.bfloat16
NPBF16 = mybir.dt.np(mybir.dt.bfloat16)

B, S, D = 2, 2048, 1536
H, DQ, DK, DV = 8, 128, 128, 192
P = 128
SQ = S // 4          # 512 query rows per core
DC = D // P          # 12 contraction chunks
JC = S // P          # 16 key chunks
SC = SQ // P         # 4 query-row chunks
NCORES = 8
EPS_RMS = 1e-6
EPS_LN = 1e-5
SOFTCAP = 5.0
ROPE_BASE = 8192.0
HALF = DQ // 2
KV = DK + DV         # 320: joint k|v row width
VP = 200             # v row: 192 v + ones col @192 + 7 pad
HG = 4               # heads per q-projection group

# consts packing (col offsets in the [128, CW] const tensor)
C_ID = 0             # ident f32 [128]
C_BKV = 128          # kv eviction bias rep [320]
C_BQ = 448           # q eviction bias rep [1024]
C_BO = 1472          # output bias rep [1536]
C_INV = 3008         # [1/DK, 1/DV] rep
C_EPS = 3010         # EPS_LN
CW = 3012


def _r(ap):
    return ap.bitcast(F32R)


def build_program(has_kb=False, has_qb=False):
    nc = bacc.Bacc(
        "TRN2", target_bir_lowering=False, debug=False, num_devices=NCORES
    )

    def din(name, shape, dt=F32):
        return nc.dram_tensor(name, list(shape), dt, kind="ExternalInput").ap()

    xT = din("xT", (D, S))                    # per-core batch
    xq_in = din("xq", (D, SQ))                # per-core q column slice
    biasT = din("biasT", (S, SQ), BF16)       # per-core, x0.2 prescaled
    ropeq = din("ropeq", (SQ, (3 if has_qb else 2) * 4 * P), BF16)  # per-core
    ropek = din("ropek", (S, (3 if has_kb else 2) * P), BF16)
    wkv = din("wkv", (D, KV))
    wq = din("wq", (D, H * DQ))
    wo = din("wo", (H * DV, D), BF16)
    consts = din("consts", (P, CW))
    out = nc.dram_tensor("out", [SQ, D], F32, kind="ExternalOutput").ap()

    TT = mybir.AluOpType
    AF = mybir.ActivationFunctionType
    AX = mybir.AxisListType

    qjc0 = None  # chunks covered by this core's q slice: set via partition id?
    # Each core's q slice differs, but the PROGRAM is shared across cores.
    # xq is just a slice of xT columns; we DMA those 4 chunks twice (once into
    # the stream tile for kv, once into xq).  Cheap (3.1MB extra DMA).

    with tile.TileContext(nc) as tc, ExitStack() as ctx:
        const = ctx.enter_context(tc.tile_pool(name="const", bufs=1))
        persist = ctx.enter_context(tc.tile_pool(name="persist", bufs=1))

        cst = const.tile([P, CW], F32)
        nc.sync.dma_start(cst[:], consts)
        ident = cst[:, C_ID : C_ID + P]
        ident_bf = const.tile([P, P], BF16)
        nc.vector.tensor_copy(ident_bf[:], ident)
        eps_ap = cst[:, C_EPS : C_EPS + 1]

        kT_sb = persist.tile([P, JC, P], F32)         # [dk, jc, key]
        vrow_sb = persist.tile([P, JC, VP], BF16)     # [key, jc, v|1|pad]
        qT_sb = persist.tile([P, H, SQ], F32)         # [dq, h, q]
        yatt_sb = persist.tile([P, SC, H * DV], BF16)  # [q, sc, hdv]

        nc.vector.memset(vrow_sb[:, :, DV : DV + 1], 1.0)
        nc.vector.memset(vrow_sb[:, :, DV + 1 :], 0.0)

        qres_cm = tc.tile_pool(name="qres", bufs=1)
        qres = qres_cm.__enter__()
        xq_sb = qres.tile([P, DC, SQ], F32)

        # =====================================================
        # Phase KV
        # =====================================================
        with (
            tc.tile_pool(name="kvs", bufs=1) as kvs,
            tc.tile_pool(name="kvw", bufs=2) as kvw,
            tc.tile_pool(name="kn4p", bufs=2) as kn4p,
            tc.tile_pool(name="kvps", bufs=2, space="PSUM") as kvps,
            tc.tile_pool(name="kvtp", bufs=2, space="PSUM") as kvtp,
        ):
            wkv_sb = kvs.tile([P, DC, KV], F32)
            nc.sync.dma_start(
                _r(wkv_sb[:]), _r(wkv.rearrange("(c p) f -> p c f", p=P))
            )
            ropek_sb = kvs.tile([P, JC, (3 if has_kb else 2) * P], BF16)
            nc.sync.dma_start(
                ropek_sb[:], ropek.rearrange("(j p) f -> p j f", p=P)
            )
            x3 = xT.rearrange("(c p) s -> p c s", p=P)

            invn = cst[:, C_INV : C_INV + 2]

            kn4 = None
            for jc in range(JC):
                if jc % 4 == 0:
                    kn4 = kn4p.tile([P, 4, P], F32, tag="kn4")
                xc = kvw.tile([P, DC, P], F32, tag="xc")
                nc.sync.dma_start(
                    _r(xc[:]), _r(x3[:, :, jc * P : (jc + 1) * P])
                )
                kv_ps = kvps.tile([P, KV], F32, tag="kv_ps")
                for dc in range(DC):
                    nc.tensor.matmul(
                        kv_ps[:],
                        _r(xc[:, dc, :]),
                        _r(wkv_sb[:, dc, :]),
                        start=(dc == 0),
                        stop=(dc == DC - 1),
                    )
                # evict + folded rms1 bias
                kvr = kvw.tile([P, KV], F32, tag="kvr")
                nc.vector.tensor_tensor(
                    kvr[:], kv_ps[:], cst[:, C_BKV : C_BKV + KV], TT.add
                )
                # LN stats for k (0:128) and v (128:320)
                sq = kvw.tile([P, KV], F32, tag="sq")
                nc.scalar.square(sq[:], kvr[:])
                st = kvw.tile([P, 12], F32, tag="st")
                nc.vector.tensor_reduce(st[:, 0:1], kvr[:, :DK], AX.X, TT.add)
                nc.vector.tensor_reduce(st[:, 1:2], kvr[:, DK:], AX.X, TT.add)
                nc.vector.tensor_reduce(st[:, 2:3], sq[:, :DK], AX.X, TT.add)
                nc.vector.tensor_reduce(st[:, 3:4], sq[:, DK:], AX.X, TT.add)
                # smu = s1*invn ; t = s2 - smu*s1 ; var = t*invn
                nc.vector.tensor_tensor(st[:, 4:6], st[:, 0:2], invn, TT.mult)
                nc.vector.tensor_tensor(st[:, 6:8], st[:, 4:6], st[:, 0:2],
                                        TT.mult)
                nc.vector.tensor_tensor(st[:, 8:10], st[:, 2:4], st[:, 6:8],
                                        TT.subtract)
                nc.vector.tensor_tensor(st[:, 8:10], st[:, 8:10], invn,
                                        TT.mult)
                nc.scalar.activation(st[:, 10:12], st[:, 8:10], AF.Sqrt,
                                     bias=eps_ap)
                nc.vector.reciprocal(st[:, 10:12], st[:, 10:12])
                # apply: k -> kn4 slot, v -> vrow (bf16)
                nc.gpsimd.tensor_scalar(
                    kn4[:, jc % 4, :], kvr[:, :DK],
                    st[:, 4:5], st[:, 10:11], TT.subtract, TT.mult,
                )
                nc.gpsimd.tensor_scalar(
                    vrow_sb[:, jc, :DV], kvr[:, DK:],
                    st[:, 5:6], st[:, 11:12], TT.subtract, TT.mult,
                )
                if jc % 4 == 3:
                    j0 = jc - 3
                    ck = ropek_sb[:, j0 : j0 + 4, 0:P]
                    sk = ropek_sb[:, j0 : j0 + 4, P : 2 * P]
                    r1 = kvw.tile([P, 4, P], F32, tag="r1")
                    r2 = kvw.tile([P, 4, P], F32, tag="r2")
                    nc.vector.tensor_tensor(r1[:], kn4[:], ck, TT.mult)
                    nc.gpsimd.tensor_tensor(
                        r2[:, :, 0:HALF], kn4[:, :, HALF:P],
                        sk[:, :, 0:HALF], TT.mult,
                    )
                    nc.gpsimd.tensor_tensor(
                        r2[:, :, HALF:P], kn4[:, :, 0:HALF],
                        sk[:, :, HALF:P], TT.mult,
                    )
                    kr = kvw.tile([P, 4, P], F32, tag="kr")
                    nc.vector.tensor_tensor(kr[:], r1[:], r2[:], TT.add)
                    if has_kb:
                        bk = ropek_sb[:, j0 : j0 + 4, 2 * P : 3 * P]
                        nc.vector.tensor_tensor(kr[:], kr[:], bk, TT.add)
                    for t in range(4):
                        scr = kvtp.tile([P, P], F32, tag="scr")
                        nc.tensor.transpose(scr[:], kr[:, t, :], ident)
                        nc.scalar.copy(_r(kT_sb[:, j0 + t, :]), scr[:])

            # prefetch q weights + tables mid-phase (SBUF timing)
            wq_sb = qres.tile([P, DC, H * DQ], F32)
            nc.sync.dma_start(
                _r(wq_sb[:]), _r(wq.rearrange("(c p) f -> p c f", p=P))
            )
            ropeq_sb = qres.tile([P, SC, (3 if has_qb else 2) * 4 * P], BF16)
            nc.sync.dma_start(
                ropeq_sb[:], ropeq.rearrange("(s p) f -> p s f", p=P)
            )

        # q slice of x (this core's own rows), one transfer
        nc.sync.dma_start(
            _r(xq_sb[:]), _r(xq_in.rearrange("(c p) s -> p c s", p=P))
        )

        # =====================================================
        # Phase Q  (two groups of 4 heads)
        # =====================================================
        with (
            tc.tile_pool(name="qw", bufs=2) as qw,
            tc.tile_pool(name="qps", bufs=2, space="PSUM") as qps,
            tc.tile_pool(name="qtp", bufs=2, space="PSUM") as qtp,
        ):
            for g in range(2):
                f0 = g * HG * DQ
                for ic in range(SC):
                    q_ps = qps.tile([P, HG * DQ], F32, tag="q_ps")
                    for dc in range(DC):
                        nc.tensor.matmul(
                            q_ps[:],
                            _r(xq_sb[:, dc, ic * P : (ic + 1) * P]),
                            _r(wq_sb[:, dc, f0 : f0 + HG * DQ]),
                            start=(dc == 0),
                            stop=(dc == DC - 1),
                        )
                    qr = qw.tile([P, HG * DQ], F32, tag="qr")
                    nc.vector.tensor_tensor(
                        qr[:], q_ps[:],
                        cst[:, C_BQ + f0 : C_BQ + f0 + HG * DQ], TT.add
                    )
                    sqq = qw.tile([P, HG * DQ], F32, tag="sqq")
                    nc.scalar.square(sqq[:], qr[:])
                    stq = qw.tile([P, 24], F32, tag="stq")
                    qr3 = qr[:].rearrange("p (h f) -> p h f", h=HG)
                    sq3 = sqq[:].rearrange("p (h f) -> p h f", h=HG)
                    nc.vector.tensor_reduce(stq[:, 0:4], qr3, AX.X, TT.add)
                    nc.vector.tensor_reduce(stq[:, 4:8], sq3, AX.X, TT.add)
                    nc.vector.tensor_scalar(
                        stq[:, 8:12], stq[:, 0:4], 1.0 / DQ, None, TT.mult
                    )
                    nc.vector.tensor_tensor(
                        stq[:, 12:16], stq[:, 8:12], stq[:, 0:4], TT.mult
                    )
                    nc.vector.tensor_tensor(
                        stq[:, 16:20], stq[:, 4:8], stq[:, 12:16], TT.subtract
                    )
                    nc.vector.tensor_scalar(
                        stq[:, 16:20], stq[:, 16:20], 1.0 / DQ, None, TT.mult
                    )
                    nc.scalar.activation(
                        stq[:, 20:24], stq[:, 16:20], AF.Sqrt, bias=eps_ap
                    )
                    nc.vector.reciprocal(stq[:, 20:24], stq[:, 20:24])
                    qn4 = qw.tile([P, HG, DQ], F32, tag="qn4")
                    for t in range(HG):
                        nc.gpsimd.tensor_scalar(
                            qn4[:, t, :], qr[:, t * DQ : (t + 1) * DQ],
                            stq[:, 8 + t : 9 + t], stq[:, 20 + t : 21 + t],
                            TT.subtract, TT.mult,
                        )
                    cq = ropeq_sb[:, ic, 0 : HG * P].rearrange(
                        "p (h f) -> p h f", h=HG)
                    sqt = ropeq_sb[:, ic, HG * P : 2 * HG * P].rearrange(
                        "p (h f) -> p h f", h=HG)
                    r1 = qw.tile([P, HG, DQ], F32, tag="qr1")
                    r2 = qw.tile([P, HG, DQ], F32, tag="qr2")
                    nc.vector.tensor_tensor(r1[:], qn4[:], cq, TT.mult)
                    nc.gpsimd.tensor_tensor(
                        r2[:, :, 0:HALF], qn4[:, :, HALF:DQ],
                        sqt[:, :, 0:HALF], TT.mult,
                    )
                    nc.gpsimd.tensor_tensor(
                        r2[:, :, HALF:DQ], qn4[:, :, 0:HALF],
                        sqt[:, :, HALF:DQ], TT.mult,
                    )
                    qrope = qw.tile([P, HG, DQ], F32, tag="qrope")
                    nc.vector.tensor_tensor(qrope[:], r1[:], r2[:], TT.add)
                    if has_qb:
                        bq4 = ropeq_sb[:, ic, 2 * HG * P : 3 * HG * P
                                       ].rearrange("p (h f) -> p h f", h=HG)
                        nc.vector.tensor_tensor(qrope[:], qrope[:], bq4,
                                                TT.add)
                    for t in range(HG):
                        scr = qtp.tile([P, P], F32, tag="qscr")
                        nc.tensor.transpose(scr[:], qrope[:, t, :], ident)
                        nc.scalar.copy(
                            _r(qT_sb[:, g * HG + t, ic * P : (ic + 1) * P]),
                            scr[:],
                        )

        qres_cm.__exit__(None, None, None)

        # =====================================================
        # Attention
        # =====================================================
        with tc.tile_pool(name="wop", bufs=1) as wop:
            biasT_sb = wop.tile([P, JC, SQ], BF16)
            nc.sync.dma_start(
                biasT_sb[:], biasT.rearrange("(j p) i -> p j i", p=P)
            )
            wo_sb = wop.tile([P, DC, D], BF16)
            nc.sync.dma_start(
                wo_sb[:], wo.rearrange("(c p) f -> p c f", p=P)
            )

            with (
                tc.tile_pool(name="att", bufs=2) as att,
                tc.tile_pool(name="apsum", bufs=2, space="PSUM") as aps,
                tc.tile_pool(name="ypsum", bufs=1, space="PSUM") as yps,
            ):
                y_ps = [
                    yps.tile([P, VP], F32, tag=f"y{ic}", name=f"y{ic}")
                    for ic in range(SC)
                ]
                for h in range(H):
                    for jp in range(JC // 4):      # pairs of 2-chunk groups
                        zt4 = att.tile([P, 4, SQ], BF16, tag="zt4")
                        for half in range(2):
                            jg = jp * 2 + half
                            pq = aps.tile([P, 2, SQ], F32, tag="pq")
                            for c in range(2):
                                jc = jg * 2 + c
                                nc.tensor.matmul(
                                    pq[:, c, :],
                                    _r(kT_sb[:, jc, :]),
                                    _r(qT_sb[:, h, :]),
                                    start=True, stop=True,
                                )
                            nc.vector.scalar_tensor_tensor(
                                zt4[:, half * 2 : half * 2 + 2, :],
                                pq[:],
                                1.0 / SOFTCAP,
                                biasT_sb[:, jg * 2 : jg * 2 + 2, :],
                                TT.mult, TT.add,
                            )
                        tt4 = att.tile([P, 4, SQ], BF16, tag="tt4")
                        nc.scalar.activation(tt4[:], zt4[:], AF.Tanh)
                        pt4 = att.tile([P, 4, SQ], BF16, tag="pt4")
                        nc.scalar.activation(pt4[:], tt4[:], AF.Exp,
                                             scale=SOFTCAP)
                        for c in range(4):
                            jc = jp * 4 + c
                            for ic in range(SC):
                                nc.tensor.matmul(
                                    y_ps[ic][:],
                                    pt4[:, c, ic * P : (ic + 1) * P],
                                    vrow_sb[:, jc, :],
                                    start=(jc == 0),
                                    stop=(jc == JC - 1),
                                )
                    for ic in range(SC):
                        recip = att.tile([P, 1], F32, tag="recip")
                        nc.vector.reciprocal(
                            recip[:], y_ps[ic][:, DV : DV + 1]
                        )
                        nc.vector.tensor_scalar(
                            yatt_sb[:, ic, h * DV : (h + 1) * DV],
                            y_ps[ic][:, :DV],
                            recip[:, 0:1], None, TT.mult,
                        )

            # =================================================
            # Output projection
            # =================================================
            with (
                tc.tile_pool(name="op", bufs=2) as op,
                tc.tile_pool(name="opsum", bufs=1, space="PSUM") as ops,
                tc.tile_pool(name="otps", bufs=2, space="PSUM") as otps,
            ):
                for sc in range(SC):
                    yT = op.tile([P, DC, P], BF16, tag="yT")
                    for fc in range(DC):
                        pt2 = otps.tile([P, P], BF16, tag="yt")
                        nc.tensor.transpose(
                            pt2[:],
                            yatt_sb[:, sc, fc * P : (fc + 1) * P],
                            ident_bf[:],
                        )
                        nc.scalar.copy(yT[:, fc, :], pt2[:])
                    o_ps = [
                        ops.tile([P, 512], F32, tag=f"o{n}", name=f"o{n}_{sc}")
                        for n in range(3)
                    ]
                    for fc in range(DC):
                        for n in range(3):
                            nc.tensor.matmul(
                                o_ps[n][:],
                                yT[:, fc, :],
                                wo_sb[:, fc, n * 512 : (n + 1) * 512],
                                start=(fc == 0),
                                stop=(fc == DC - 1),
                            )
                    o_sb = op.tile([P, D], F32, tag="o_sb")
                    for n in range(3):
                        nc.vector.tensor_tensor(
                            o_sb[:, n * 512 : (n + 1) * 512],
                            o_ps[n][:],
                            cst[:, C_BO + n * 512 : C_BO + (n + 1) * 512],
                            TT.add,
                        )
                    nc.sync.dma_start(out[sc * P : (sc + 1) * P, :], o_sb[:])

    nc.compile()
    return nc


def _rope_tables(n, g, b, scale, start=0):
    """Full-width tables (f32): out = xhat*C + xswap*Sw (+ B)."""
    f32 = np.float32
    freqs = 1.0 / (ROPE_BASE ** (np.arange(HALF, dtype=f32) / HALF))
    ang = (start + np.arange(n, dtype=f32))[:, None] * freqs[None, :]
    cos, sin = np.cos(ang).astype(f32), np.sin(ang).astype(f32)
    g1, g2 = g[:HALF], g[HALF:]
    b1, b2 = b[:HALF], b[HALF:]
    C = np.concatenate([g1 * cos, g2 * cos], axis=1) * scale
    Sw = np.concatenate([-g2 * sin, g1 * sin], axis=1) * scale
    Bt = np.concatenate([b1 * cos - b2 * sin, b1 * sin + b2 * cos],
                        axis=1) * scale
    return C.astype(f32), Sw.astype(f32), Bt.astype(f32)


def _host_prep(inputs):
    f32 = np.float32
    x = np.asarray(inputs["x"], f32)
    bias = np.asarray(inputs["attention_bias"], f32)
    g1 = np.asarray(inputs["g1"], f32)
    b1 = np.asarray(inputs["b1"], f32)
    rr1 = np.asarray(inputs["rrms1"], f32)
    Wq = np.asarray(inputs["Wq"], f32)
    Wk = np.asarray(inputs["Wk"], f32)
    Wv = np.asarray(inputs["Wv"], f32)
    qg = np.asarray(inputs["qg"], f32)
    qb = np.asarray(inputs["qb"], f32)
    kg = np.asarray(inputs["kg"], f32)
    kb = np.asarray(inputs["kb"], f32)
    vg = np.asarray(inputs["vg"], f32)
    vb = np.asarray(inputs["vb"], f32)
    Wo = np.asarray(inputs["Wo"], f32)
    bo = np.asarray(inputs["bo"], f32)
    g2 = np.asarray(inputs["g2"], f32)
    b2 = np.asarray(inputs["b2"], f32)
    rr2 = np.asarray(inputs["rrms2"], f32)

    has_kb = bool(np.any(kb != 0))
    has_qb = bool(np.any(qb != 0))

    scale1 = (g1 * (1.0 / np.sqrt(rr1 + EPS_RMS))).astype(f32)
    Wkv = np.concatenate([Wk * scale1[:, None], Wv * scale1[:, None]],
                         axis=1).astype(f32)
    bkv = np.concatenate([b1 @ Wk, b1 @ Wv]).astype(f32)
    Wq_e = (Wq * scale1[:, None]).astype(f32)
    bq_row = (b1 @ Wq).astype(f32)

    sc_q = f32(DQ) ** f32(-0.5)
    scale2 = (g2 * (1.0 / np.sqrt(rr2 + EPS_RMS))).astype(f32)
    vg_t = np.tile(vg, H).astype(f32)
    vb_t = np.tile(vb, H).astype(f32)
    Wo_f = (vg_t[:, None] * Wo * scale2[None, :]).astype(f32)
    bo_f = ((vb_t @ Wo + bo) * scale2 + b2).astype(f32)

    # k rope tables (full S)
    Ck, Sk, Bk = _rope_tables(S, kg, kb, f32(1.0))
    ropek_cols = [Ck, Sk] + ([Bk] if has_kb else [])
    ropek = np.concatenate(ropek_cols, axis=1).astype(NPBF16)

    rep = lambda v: np.ascontiguousarray(
        np.broadcast_to(v[None, :], (P, v.shape[0])), dtype=f32)
    consts = np.zeros((P, CW), f32)
    consts[:, C_ID : C_ID + P] = np.eye(P, dtype=f32)
    consts[:, C_BKV : C_BKV + KV] = rep(bkv)
    consts[:, C_BQ : C_BQ + H * DQ] = rep(bq_row)
    consts[:, C_BO : C_BO + D] = rep(bo_f)
    consts[:, C_INV] = f32(1.0 / DK)
    consts[:, C_INV + 1] = f32(1.0 / DV)
    consts[:, C_EPS] = f32(EPS_LN)

    shared = {
        "wkv": np.ascontiguousarray(Wkv),
        "wq": np.ascontiguousarray(Wq_e),
        "wo": np.ascontiguousarray(Wo_f.astype(NPBF16)),
        "ropek": np.ascontiguousarray(ropek),
        "consts": np.ascontiguousarray(consts),
    }

    xTs = [np.ascontiguousarray(x[b].T) for b in range(B)]
    in_maps = []
    for c in range(NCORES):
        b = c // 4
        s0 = (c % 4) * SQ
        m = dict(shared)
        m["xT"] = xTs[b]
        m["xq"] = np.ascontiguousarray(xTs[b][:, s0 : s0 + SQ])
        m["biasT"] = np.ascontiguousarray(
            (bias[0, 0, s0 : s0 + SQ, :].T * (1.0 / SOFTCAP)).astype(NPBF16)
        )
        Cq, Sq, Bq = _rope_tables(SQ, qg, qb, sc_q, start=s0)
        rq_cols = [np.tile(Cq, (1, HG)), np.tile(Sq, (1, HG))]
        if has_qb:
            rq_cols.append(np.tile(Bq, (1, HG)))
        m["ropeq"] = np.ascontiguousarray(
            np.concatenate(rq_cols, axis=1).astype(NPBF16)
        )
        in_maps.append(m)
    return in_maps, (has_kb, has_qb)


_NC_CACHE = {}


def _get_nc(flags=(False, False)):
    if flags not in _NC_CACHE:
        _NC_CACHE[flags] = build_program(*flags)
    return _NC_CACHE[flags]


def kernel(**inputs) -> np.ndarray:
    in_maps, flags = _host_prep(inputs)
    nc = _get_nc(flags)
    res = bass_utils.run_bass_kernel_spmd(
        nc, in_maps, core_ids=list(range(NCORES))
    )
    outs = res.results
    full = np.empty((B, S, D), np.float32)
    for c in range(NCORES):
        b = c // 4
        s0 = (c % 4) * SQ
        full[b, s0 : s0 + SQ, :] = outs[c]["out"]
    return full


if __name__ == "__main__":
    nc = _get_nc()
    print("build + compile OK")


# revision 17
# speedup vs baseline: 1.8727x; 1.2482x over previous
"""Trainium2 Bass kernel for an MQA attention block (8 q-heads, shared K/V).

Sharding: 8 cores; core c -> batch b=c//4, query rows s0=(c%4)*512 .. +512,
all 8 heads.  K/V (full sequence, per batch) are computed redundantly on each
core; no cross-core communication.

Layout strategy ("row" architecture):
 - K/V projection computed jointly in ROW layout (keys on partitions): per
   key-chunk, lhsT = xT chunk (stationary), rhs = [Wk|Wv] (moving, N=320).
   LayerNorm stats are per-partition; affines are folded into rope tables
   (k) and into Wo/bo (v).  v rows written bf16, consumed directly by the
   attention y-matmul; k rows rope'd then PE-transposed to kT.
 - Q projection in ROW layout per half-head-group (N=512), LN+rope in rows
   (affine + DQ^-0.5 folded into tables), PE-transposed to qT.
 - Attention: logits = kT.T @ qT in fp32r; softcap chain is one DVE
   scalar_tensor_tensor (x0.2 + bias, writes bf16) + ACT tanh + ACT exp
   (both bf16, 2x rate); y accumulates p.T @ [v|1|pad] (bf16, N=200) with
   the ones-column giving the softmax denominator.
 - Output projection: yatt rows (bf16) -> PE transpose (bf16) -> yT,
   matmul with bf16 Wo (v-affine + rms2 folded), add bias, DMA out.
"""

import os
import sys

for _p in ("/opt/trn_rl_repo",):
    if _p not in sys.path and os.path.isdir(_p):
        sys.path.insert(0, _p)

import numpy as np
from contextlib import ExitStack

import concourse.bass as bass
import concourse.mybir as mybir
import concourse.tile as tile
from concourse import bacc
from concourse import bass_utils

F32 = mybir.dt.float32
F32R = mybir.dt.float32r
BF16 = mybir.dt.bfloat16
NPBF16 = mybir.dt.np(mybir.dt.bfloat16)

B, S, D = 2, 2048, 1536
H, DQ, DK, DV = 8, 128, 128, 192
P = 128
SQ = S // 4          # 512 query rows per core
DC = D // P          # 12 contraction chunks
JC = S // P          # 16 key chunks
SC = SQ // P         # 4 query-row chunks
NCORES = 8
EPS_RMS = 1e-6
EPS_LN = 1e-5
SOFTCAP = 5.0
ROPE_BASE = 8192.0
HALF = DQ // 2
KV = DK + DV         # 320: joint k|v row width
VP = 200             # v row: 192 v + ones col @192 + 7 pad
HG = 4               # heads per q-projection group

# consts packing (col offsets in the [128, CW] const tensor)
C_ID = 0             # ident f32 [128]
C_BKV = 128          # kv eviction bias rep [320]
C_BQ = 448           # q eviction bias rep [1024]
C_BO = 1472          # output bias rep [1536]
C_INV = 3008         # [1/DK, 1/DV] rep
C_EPS = 3010         # EPS_LN
CW = 3012


def _r(ap):
    return ap.bitcast(F32R)


def build_program(has_kb=False, has_qb=False):
    nc = bacc.Bacc(
        "TRN2", target_bir_lowering=False, debug=False, num_devices=NCORES
    )

    def din(name, shape, dt=F32):
        return nc.dram_tensor(name, list(shape), dt, kind="ExternalInput").ap()

    xT = din("xT", (D, S))                    # per-core batch
    xq_in = din("xq", (D, SQ))                # per-core q column slice
    biasT = din("biasT", (S, SQ), BF16)       # per-core, x0.2 prescaled
    ropeq = din("ropeq", (SQ, (3 if has_qb else 2) * 4 * P), BF16)  # per-core
    ropek = din("ropek", (S, (3 if has_kb else 2) * P), BF16)
    wkv = din("wkv", (D, KV))
    wq = din("wq", (D, H * DQ))
    wo = din("wo", (H * DV, D), BF16)
    consts = din("consts", (P, CW))
    out = nc.dram_tensor("out", [SQ, D], F32, kind="ExternalOutput").ap()

    TT = mybir.AluOpType
    AF = mybir.ActivationFunctionType
    AX = mybir.AxisListType

    qjc0 = None  # chunks covered by this core's q slice: set via partition id?
    # Each core's q slice differs, but the PROGRAM is shared across cores.
    # xq is just a slice of xT columns; we DMA those 4 chunks twice (once into
    # the stream tile for kv, once into xq).  Cheap (3.1MB extra DMA).

    with tile.TileContext(nc) as tc, ExitStack() as ctx:
        const = ctx.enter_context(tc.tile_pool(name="const", bufs=1))
        persist = ctx.enter_context(tc.tile_pool(name="persist", bufs=1))

        cst = const.tile([P, CW], F32)
        nc.sync.dma_start(cst[:], consts)
        ident = cst[:, C_ID : C_ID + P]
        ident_bf = const.tile([P, P], BF16)
        nc.vector.tensor_copy(ident_bf[:], ident)
        eps_ap = cst[:, C_EPS : C_EPS + 1]

        kT_sb = persist.tile([P, JC, P], F32)         # [dk, jc, key]
        vrow_sb = persist.tile([P, JC, VP], BF16)     # [key, jc, v|1|pad]
        qT_sb = persist.tile([P, H, SQ], F32)         # [dq, h, q]

        nc.vector.memset(vrow_sb[:, :, DV : DV + 1], 1.0)
        nc.vector.memset(vrow_sb[:, :, DV + 1 :], 0.0)

        qres_cm = tc.tile_pool(name="qres", bufs=1)
        qres = qres_cm.__enter__()
        xq_sb = qres.tile([P, DC, SQ], F32)

        # =====================================================
        # Phase KV
        # =====================================================
        with (
            tc.tile_pool(name="kvs", bufs=1) as kvs,
            tc.tile_pool(name="kvw", bufs=2) as kvw,
            tc.tile_pool(name="kn4p", bufs=2) as kn4p,
            tc.tile_pool(name="kvps", bufs=2, space="PSUM") as kvps,
            tc.tile_pool(name="kvtp", bufs=2, space="PSUM") as kvtp,
        ):
            wkv_sb = kvs.tile([P, DC, KV], F32)
            nc.sync.dma_start(
                _r(wkv_sb[:]), _r(wkv.rearrange("(c p) f -> p c f", p=P))
            )
            ropek_sb = kvs.tile([P, JC, (3 if has_kb else 2) * P], BF16)
            nc.sync.dma_start(
                ropek_sb[:], ropek.rearrange("(j p) f -> p j f", p=P)
            )
            x3 = xT.rearrange("(c p) s -> p c s", p=P)

            invn = cst[:, C_INV : C_INV + 2]

            kn4 = None
            xslab = None
            for jc in range(JC):
                if jc % 4 == 0:
                    kn4 = kn4p.tile([P, 4, P], F32, tag="kn4")
                if jc % 2 == 0:
                    xslab = kvw.tile([P, DC, 2 * P], F32, tag="xslab")
                    nc.sync.dma_start(
                        _r(xslab[:]),
                        _r(x3[:, :, jc * P : (jc + 2) * P]),
                    )
                xc = xslab[:, :, (jc % 2) * P : (jc % 2 + 1) * P]
                kv_ps = kvps.tile([P, KV], F32, tag="kv_ps")
                for dc in range(DC):
                    nc.tensor.matmul(
                        kv_ps[:],
                        _r(xc[:, dc, :]),
                        _r(wkv_sb[:, dc, :]),
                        start=(dc == 0),
                        stop=(dc == DC - 1),
                    )
                # evict + folded rms1 bias; accum gives the LN sums free
                kvr = kvw.tile([P, KV], F32, tag="kvr")
                st = kvw.tile([P, 16], F32, tag="st")
                nc.vector.scalar_tensor_tensor(
                    kvr[:, :DK], kv_ps[:, :DK], 1.0,
                    cst[:, C_BKV : C_BKV + DK], TT.mult, TT.add,
                    accum_out=st[:, 0:1],
                )
                nc.vector.scalar_tensor_tensor(
                    kvr[:, DK:], kv_ps[:, DK:], 1.0,
                    cst[:, C_BKV + DK : C_BKV + KV], TT.mult, TT.add,
                    accum_out=st[:, 1:2],
                )
                # sumsq on ACT (square with accumulator)
                sq = kvw.tile([P, KV], F32, tag="sq")
                nc.scalar.activation(sq[:, :DK], kvr[:, :DK], AF.Square,
                                     accum_out=st[:, 2:3])
                nc.scalar.activation(sq[:, DK:], kvr[:, DK:], AF.Square,
                                     accum_out=st[:, 3:4])
                # smu = s1*invn ; var = (s2 - smu*s1)*invn ; rs = rsqrt(var+eps)
                nc.vector.tensor_tensor(st[:, 4:6], st[:, 0:2], invn, TT.mult)
                nc.vector.tensor_tensor(st[:, 6:8], st[:, 4:6], st[:, 0:2],
                                        TT.mult)
                nc.vector.tensor_tensor(st[:, 8:10], st[:, 2:4], st[:, 6:8],
                                        TT.subtract)
                nc.vector.tensor_tensor(st[:, 8:10], st[:, 8:10], invn,
                                        TT.mult)
                nc.scalar.activation(st[:, 10:12], st[:, 8:10], AF.Sqrt,
                                     bias=eps_ap)
                nc.vector.reciprocal(st[:, 10:12], st[:, 10:12])
                # nmr = -smu*rs  (bias for the ACT-side LN apply)
                nc.vector.scalar_tensor_tensor(
                    st[:, 12:14], st[:, 4:6], -1.0, st[:, 10:12],
                    TT.mult, TT.mult,
                )
                # apply (ACT): out = in*rs + (-smu*rs)
                nc.scalar.activation(
                    kn4[:, jc % 4, :], kvr[:, :DK], AF.Identity,
                    bias=st[:, 12:13], scale=st[:, 10:11],
                )
                nc.scalar.activation(
                    vrow_sb[:, jc, :DV], kvr[:, DK:], AF.Identity,
                    bias=st[:, 13:14], scale=st[:, 11:12],
                )
                if jc % 4 == 3:
                    j0 = jc - 3
                    ck = ropek_sb[:, j0 : j0 + 4, 0:P]
                    sk = ropek_sb[:, j0 : j0 + 4, P : 2 * P]
                    r1 = kvw.tile([P, 4, P], F32, tag="r1")
                    r2 = kvw.tile([P, 4, P], F32, tag="r2")
                    nc.vector.tensor_tensor(r1[:], kn4[:], ck, TT.mult)
                    nc.gpsimd.tensor_tensor(
                        r2[:, :, 0:HALF], kn4[:, :, HALF:P],
                        sk[:, :, 0:HALF], TT.mult,
                    )
                    nc.gpsimd.tensor_tensor(
                        r2[:, :, HALF:P], kn4[:, :, 0:HALF],
                        sk[:, :, HALF:P], TT.mult,
                    )
                    kr = kvw.tile([P, 4, P], F32, tag="kr")
                    nc.vector.tensor_tensor(kr[:], r1[:], r2[:], TT.add)
                    if has_kb:
                        bk = ropek_sb[:, j0 : j0 + 4, 2 * P : 3 * P]
                        nc.vector.tensor_tensor(kr[:], kr[:], bk, TT.add)
                    for t in range(4):
                        scr = kvtp.tile([P, P], F32, tag="scr")
                        nc.tensor.transpose(scr[:], kr[:, t, :], ident)
                        nc.scalar.copy(_r(kT_sb[:, j0 + t, :]), scr[:])

            # prefetch q weights + tables mid-phase (SBUF timing)
            wq_sb = qres.tile([P, DC, H * DQ], F32)
            nc.sync.dma_start(
                _r(wq_sb[:]), _r(wq.rearrange("(c p) f -> p c f", p=P))
            )
            ropeq_sb = qres.tile([P, SC, (3 if has_qb else 2) * 4 * P], BF16)
            nc.sync.dma_start(
                ropeq_sb[:], ropeq.rearrange("(s p) f -> p s f", p=P)
            )

        # q slice of x (this core's own rows), one transfer
        nc.sync.dma_start(
            _r(xq_sb[:]), _r(xq_in.rearrange("(c p) s -> p c s", p=P))
        )

        # =====================================================
        # Phase Q  (two groups of 4 heads)
        # =====================================================
        with (
            tc.tile_pool(name="qw", bufs=2) as qw,
            tc.tile_pool(name="qps", bufs=2, space="PSUM") as qps,
            tc.tile_pool(name="qtp", bufs=2, space="PSUM") as qtp,
        ):
            for g in range(2):
                f0 = g * HG * DQ
                for ic in range(SC):
                    q_ps = qps.tile([P, HG * DQ], F32, tag="q_ps")
                    for dc in range(DC):
                        nc.tensor.matmul(
                            q_ps[:],
                            _r(xq_sb[:, dc, ic * P : (ic + 1) * P]),
                            _r(wq_sb[:, dc, f0 : f0 + HG * DQ]),
                            start=(dc == 0),
                            stop=(dc == DC - 1),
                        )
                    qr = qw.tile([P, HG * DQ], F32, tag="qr")
                    nc.vector.tensor_tensor(
                        qr[:], q_ps[:],
                        cst[:, C_BQ + f0 : C_BQ + f0 + HG * DQ], TT.add
                    )
                    sqq = qw.tile([P, HG * DQ], F32, tag="sqq")
                    stq = qw.tile([P, 28], F32, tag="stq")
                    qr3 = qr[:].rearrange("p (h f) -> p h f", h=HG)
                    nc.vector.tensor_reduce(stq[:, 0:4], qr3, AX.X, TT.add)
                    for t in range(HG):
                        nc.scalar.activation(
                            sqq[:, t * DQ : (t + 1) * DQ],
                            qr[:, t * DQ : (t + 1) * DQ],
                            AF.Square, accum_out=stq[:, 4 + t : 5 + t],
                        )
                    nc.vector.tensor_scalar(
                        stq[:, 8:12], stq[:, 0:4], 1.0 / DQ, None, TT.mult
                    )
                    nc.vector.tensor_tensor(
                        stq[:, 12:16], stq[:, 8:12], stq[:, 0:4], TT.mult
                    )
                    nc.vector.tensor_tensor(
                        stq[:, 16:20], stq[:, 4:8], stq[:, 12:16], TT.subtract
                    )
                    nc.vector.tensor_scalar(
                        stq[:, 16:20], stq[:, 16:20], 1.0 / DQ, None, TT.mult
                    )
                    nc.scalar.activation(
                        stq[:, 20:24], stq[:, 16:20], AF.Sqrt, bias=eps_ap
                    )
                    nc.vector.reciprocal(stq[:, 20:24], stq[:, 20:24])
                    nc.vector.scalar_tensor_tensor(
                        stq[:, 24:28], stq[:, 8:12], -1.0, stq[:, 20:24],
                        TT.mult, TT.mult,
                    )
                    qn4 = qw.tile([P, HG, DQ], F32, tag="qn4")
                    for t in range(HG):
                        nc.scalar.activation(
                            qn4[:, t, :], qr[:, t * DQ : (t + 1) * DQ],
                            AF.Identity,
                            bias=stq[:, 24 + t : 25 + t],
                            scale=stq[:, 20 + t : 21 + t],
                        )
                    cq = ropeq_sb[:, ic, 0 : HG * P].rearrange(
                        "p (h f) -> p h f", h=HG)
                    sqt = ropeq_sb[:, ic, HG * P : 2 * HG * P].rearrange(
                        "p (h f) -> p h f", h=HG)
                    r1 = qw.tile([P, HG, DQ], F32, tag="qr1")
                    r2 = qw.tile([P, HG, DQ], F32, tag="qr2")
                    nc.vector.tensor_tensor(r1[:], qn4[:], cq, TT.mult)
                    nc.gpsimd.tensor_tensor(
                        r2[:, :, 0:HALF], qn4[:, :, HALF:DQ],
                        sqt[:, :, 0:HALF], TT.mult,
                    )
                    nc.gpsimd.tensor_tensor(
                        r2[:, :, HALF:DQ], qn4[:, :, 0:HALF],
                        sqt[:, :, HALF:DQ], TT.mult,
                    )
                    qrope = qw.tile([P, HG, DQ], F32, tag="qrope")
                    nc.vector.tensor_tensor(qrope[:], r1[:], r2[:], TT.add)
                    if has_qb:
                        bq4 = ropeq_sb[:, ic, 2 * HG * P : 3 * HG * P
                                       ].rearrange("p (h f) -> p h f", h=HG)
                        nc.vector.tensor_tensor(qrope[:], qrope[:], bq4,
                                                TT.add)
                    for t in range(HG):
                        scr = qtp.tile([P, P], F32, tag="qscr")
                        nc.tensor.transpose(scr[:], qrope[:, t, :], ident)
                        nc.scalar.copy(
                            _r(qT_sb[:, g * HG + t, ic * P : (ic + 1) * P]),
                            scr[:],
                        )

        qres_cm.__exit__(None, None, None)

        # =====================================================
        # Attention
        # =====================================================
        with tc.tile_pool(name="wop", bufs=1) as wop:
            yatt_sb = wop.tile([P, SC, H * DV], BF16)  # [q, sc, hdv]
            biasT_sb = wop.tile([P, JC, SQ], BF16)
            nc.sync.dma_start(
                biasT_sb[:], biasT.rearrange("(j p) i -> p j i", p=P)
            )
            wo_sb = wop.tile([P, DC, D], BF16)
            nc.sync.dma_start(
                wo_sb[:], wo.rearrange("(c p) f -> p c f", p=P)
            )

            with (
                tc.tile_pool(name="att", bufs=2) as att,
                tc.tile_pool(name="apsum", bufs=2, space="PSUM") as aps,
                tc.tile_pool(name="ypsum", bufs=1, space="PSUM") as yps,
            ):
                y_ps = [
                    yps.tile([P, VP], F32, tag=f"y{ic}", name=f"y{ic}")
                    for ic in range(SC)
                ]
                for h in range(H):
                    for jp in range(JC // 4):      # pairs of 2-chunk groups
                        zt4 = att.tile([P, 4, SQ], BF16, tag="zt4")
                        for half in range(2):
                            jg = jp * 2 + half
                            pq = aps.tile([P, 2, SQ], F32, tag="pq")
                            for c in range(2):
                                jc = jg * 2 + c
                                nc.tensor.matmul(
                                    pq[:, c, :],
                                    _r(kT_sb[:, jc, :]),
                                    _r(qT_sb[:, h, :]),
                                    start=True, stop=True,
                                )
                            nc.vector.scalar_tensor_tensor(
                                zt4[:, half * 2 : half * 2 + 2, :],
                                pq[:],
                                1.0 / SOFTCAP,
                                biasT_sb[:, jg * 2 : jg * 2 + 2, :],
                                TT.mult, TT.add,
                            )
                        tt4 = att.tile([P, 4, SQ], BF16, tag="tt4")
                        nc.scalar.activation(tt4[:], zt4[:], AF.Tanh)
                        pt4 = att.tile([P, 4, SQ], BF16, tag="pt4")
                        nc.scalar.activation(pt4[:], tt4[:], AF.Exp,
                                             scale=SOFTCAP)
                        for c in range(4):
                            jc = jp * 4 + c
                            for ic in range(SC):
                                nc.tensor.matmul(
                                    y_ps[ic][:],
                                    pt4[:, c, ic * P : (ic + 1) * P],
                                    vrow_sb[:, jc, :],
                                    start=(jc == 0),
                                    stop=(jc == JC - 1),
                                )
                    for ic in range(SC):
                        recip = att.tile([P, 1], F32, tag="recip")
                        nc.vector.reciprocal(
                            recip[:], y_ps[ic][:, DV : DV + 1]
                        )
                        nc.vector.tensor_scalar(
                            yatt_sb[:, ic, h * DV : (h + 1) * DV],
                            y_ps[ic][:, :DV],
                            recip[:, 0:1], None, TT.mult,
                        )

            # =================================================
            # Output projection
            # =================================================
            with (
                tc.tile_pool(name="op", bufs=2) as op,
                tc.tile_pool(name="opsum", bufs=1, space="PSUM") as ops,
                tc.tile_pool(name="otps", bufs=2, space="PSUM") as otps,
            ):
                for sc in range(SC):
                    yT = op.tile([P, DC, P], BF16, tag="yT")
                    for fc in range(DC):
                        pt2 = otps.tile([P, P], BF16, tag="yt")
                        nc.tensor.transpose(
                            pt2[:],
                            yatt_sb[:, sc, fc * P : (fc + 1) * P],
                            ident_bf[:],
                        )
                        nc.scalar.copy(yT[:, fc, :], pt2[:])
                    o_ps = [
                        ops.tile([P, 512], F32, tag=f"o{n}", name=f"o{n}_{sc}")
                        for n in range(3)
                    ]
                    for fc in range(DC):
                        for n in range(3):
                            nc.tensor.matmul(
                                o_ps[n][:],
                                yT[:, fc, :],
                                wo_sb[:, fc, n * 512 : (n + 1) * 512],
                                start=(fc == 0),
                                stop=(fc == DC - 1),
                            )
                    o_sb = op.tile([P, D], F32, tag="o_sb")
                    for n in range(3):
                        nc.vector.tensor_tensor(
                            o_sb[:, n * 512 : (n + 1) * 512],
                            o_ps[n][:],
                            cst[:, C_BO + n * 512 : C_BO + (n + 1) * 512],
                            TT.add,
                        )
                    nc.sync.dma_start(out[sc * P : (sc + 1) * P, :], o_sb[:])

    nc.compile()
    return nc


def _rope_tables(n, g, b, scale, start=0):
    """Full-width tables (f32): out = xhat*C + xswap*Sw (+ B)."""
    f32 = np.float32
    freqs = 1.0 / (ROPE_BASE ** (np.arange(HALF, dtype=f32) / HALF))
    ang = (start + np.arange(n, dtype=f32))[:, None] * freqs[None, :]
    cos, sin = np.cos(ang).astype(f32), np.sin(ang).astype(f32)
    g1, g2 = g[:HALF], g[HALF:]
    b1, b2 = b[:HALF], b[HALF:]
    C = np.concatenate([g1 * cos, g2 * cos], axis=1) * scale
    Sw = np.concatenate([-g2 * sin, g1 * sin], axis=1) * scale
    Bt = np.concatenate([b1 * cos - b2 * sin, b1 * sin + b2 * cos],
                        axis=1) * scale
    return C.astype(f32), Sw.astype(f32), Bt.astype(f32)


def _host_prep(inputs):
    f32 = np.float32
    x = np.asarray(inputs["x"], f32)
    bias = np.asarray(inputs["attention_bias"], f32)
    g1 = np.asarray(inputs["g1"], f32)
    b1 = np.asarray(inputs["b1"], f32)
    rr1 = np.asarray(inputs["rrms1"], f32)
    Wq = np.asarray(inputs["Wq"], f32)
    Wk = np.asarray(inputs["Wk"], f32)
    Wv = np.asarray(inputs["Wv"], f32)
    qg = np.asarray(inputs["qg"], f32)
    qb = np.asarray(inputs["qb"], f32)
    kg = np.asarray(inputs["kg"], f32)
    kb = np.asarray(inputs["kb"], f32)
    vg = np.asarray(inputs["vg"], f32)
    vb = np.asarray(inputs["vb"], f32)
    Wo = np.asarray(inputs["Wo"], f32)
    bo = np.asarray(inputs["bo"], f32)
    g2 = np.asarray(inputs["g2"], f32)
    b2 = np.asarray(inputs["b2"], f32)
    rr2 = np.asarray(inputs["rrms2"], f32)

    has_kb = bool(np.any(kb != 0))
    has_qb = bool(np.any(qb != 0))

    scale1 = (g1 * (1.0 / np.sqrt(rr1 + EPS_RMS))).astype(f32)
    Wkv = np.concatenate([Wk * scale1[:, None], Wv * scale1[:, None]],
                         axis=1).astype(f32)
    bkv = np.concatenate([b1 @ Wk, b1 @ Wv]).astype(f32)
    Wq_e = (Wq * scale1[:, None]).astype(f32)
    bq_row = (b1 @ Wq).astype(f32)

    sc_q = f32(DQ) ** f32(-0.5)
    scale2 = (g2 * (1.0 / np.sqrt(rr2 + EPS_RMS))).astype(f32)
    vg_t = np.tile(vg, H).astype(f32)
    vb_t = np.tile(vb, H).astype(f32)
    Wo_f = (vg_t[:, None] * Wo * scale2[None, :]).astype(f32)
    bo_f = ((vb_t @ Wo + bo) * scale2 + b2).astype(f32)

    # k rope tables (full S)
    Ck, Sk, Bk = _rope_tables(S, kg, kb, f32(1.0))
    ropek_cols = [Ck, Sk] + ([Bk] if has_kb else [])
    ropek = np.concatenate(ropek_cols, axis=1).astype(NPBF16)

    rep = lambda v: np.ascontiguousarray(
        np.broadcast_to(v[None, :], (P, v.shape[0])), dtype=f32)
    consts = np.zeros((P, CW), f32)
    consts[:, C_ID : C_ID + P] = np.eye(P, dtype=f32)
    consts[:, C_BKV : C_BKV + KV] = rep(bkv)
    consts[:, C_BQ : C_BQ + H * DQ] = rep(bq_row)
    consts[:, C_BO : C_BO + D] = rep(bo_f)
    consts[:, C_INV] = f32(1.0 / DK)
    consts[:, C_INV + 1] = f32(1.0 / DV)
    consts[:, C_EPS] = f32(EPS_LN)

    shared = {
        "wkv": np.ascontiguousarray(Wkv),
        "wq": np.ascontiguousarray(Wq_e),
        "wo": np.ascontiguousarray(Wo_f.astype(NPBF16)),
        "ropek": np.ascontiguousarray(ropek),
        "consts": np.ascontiguousarray(consts),
    }

    xTs = [np.ascontiguousarray(x[b].T) for b in range(B)]
    in_maps = []
    for c in range(NCORES):
        b = c // 4
        s0 = (c % 4) * SQ
        m = dict(shared)
        m["xT"] = xTs[b]
        m["xq"] = np.ascontiguousarray(xTs[b][:, s0 : s0 + SQ])
        m["biasT"] = np.ascontiguousarray(
            (bias[0, 0, s0 : s0 + SQ, :].T * (1.0 / SOFTCAP)).astype(NPBF16)
        )
        Cq, Sq, Bq = _rope_tables(SQ, qg, qb, sc_q, start=s0)
        rq_cols = [np.tile(Cq, (1, HG)), np.tile(Sq, (1, HG))]
        if has_qb:
            rq_cols.append(np.tile(Bq, (1, HG)))
        m["ropeq"] = np.ascontiguousarray(
            np.concatenate(rq_cols, axis=1).astype(NPBF16)
        )
        in_maps.append(m)
    return in_maps, (has_kb, has_qb)


_NC_CACHE = {}


def _get_nc(flags=(False, False)):
    if flags not in _NC_CACHE:
        _NC_CACHE[flags] = build_program(*flags)
    return _NC_CACHE[flags]


def kernel(**inputs) -> np.ndarray:
    in_maps, flags = _host_prep(inputs)
    nc = _get_nc(flags)
    res = bass_utils.run_bass_kernel_spmd(
        nc, in_maps, core_ids=list(range(NCORES))
    )
    outs = res.results
    full = np.empty((B, S, D), np.float32)
    for c in range(NCORES):
        b = c // 4
        s0 = (c % 4) * SQ
        full[b, s0 : s0 + SQ, :] = outs[c]["out"]
    return full


if __name__ == "__main__":
    nc = _get_nc()
    print("build + compile OK")


# revision 19
# speedup vs baseline: 1.9466x; 1.0395x over previous
"""Trainium2 Bass kernel for an MQA attention block (8 q-heads, shared K/V).

Sharding: 8 cores; core c -> batch b=c//4, query rows s0=(c%4)*512 .. +512,
all 8 heads.  K/V (full sequence, per batch) are computed redundantly on each
core; no cross-core communication.

Layout strategy ("row" architecture):
 - K/V projection computed jointly in ROW layout (keys on partitions): per
   key-chunk, lhsT = xT chunk (stationary), rhs = [Wk|Wv] (moving, N=320).
   LayerNorm stats are per-partition; affines are folded into rope tables
   (k) and into Wo/bo (v).  v rows written bf16, consumed directly by the
   attention y-matmul; k rows rope'd then PE-transposed to kT.
 - Q projection in ROW layout per half-head-group (N=512), LN+rope in rows
   (affine + DQ^-0.5 folded into tables), PE-transposed to qT.
 - Attention: logits = kT.T @ qT in fp32r; softcap chain is one DVE
   scalar_tensor_tensor (x0.2 + bias, writes bf16) + ACT tanh + ACT exp
   (both bf16, 2x rate); y accumulates p.T @ [v|1|pad] (bf16, N=200) with
   the ones-column giving the softmax denominator.
 - Output projection: yatt rows (bf16) -> PE transpose (bf16) -> yT,
   matmul with bf16 Wo (v-affine + rms2 folded), add bias, DMA out.
"""

import os
import sys

for _p in ("/opt/trn_rl_repo",):
    if _p not in sys.path and os.path.isdir(_p):
        sys.path.insert(0, _p)

import numpy as np
from contextlib import ExitStack

import concourse.bass as bass
import concourse.mybir as mybir
import concourse.tile as tile
from concourse import bacc
from concourse import bass_utils

F32 = mybir.dt.float32
F32R = mybir.dt.float32r
BF16 = mybir.dt.bfloat16
NPBF16 = mybir.dt.np(mybir.dt.bfloat16)

B, S, D = 2, 2048, 1536
H, DQ, DK, DV = 8, 128, 128, 192
P = 128
SQ = S // 4          # 512 query rows per core
DC = D // P          # 12 contraction chunks
JC = S // P          # 16 key chunks
SC = SQ // P         # 4 query-row chunks
NCORES = 8
EPS_RMS = 1e-6
EPS_LN = 1e-5
SOFTCAP = 5.0
ROPE_BASE = 8192.0
HALF = DQ // 2
KV = DK + DV         # 320: joint k|v row width
VP = 200             # v row: 192 v + ones col @192 + 7 pad
HG = 4               # heads per q-projection group

# consts packing (col offsets in the [128, CW] const tensor)
C_ID = 0             # ident f32 [128]
C_BKV = 128          # kv eviction bias rep [320]
C_BQ = 448           # q eviction bias rep [1024]
C_BO = 1472          # output bias rep [1536]
C_INV = 3008         # [1/DK, 1/DV] rep
C_EPS = 3010         # EPS_LN
CW = 3012


def _r(ap):
    return ap.bitcast(F32R)


def build_program(has_kb=False, has_qb=False):
    nc = bacc.Bacc(
        "TRN2", target_bir_lowering=False, debug=False, num_devices=NCORES
    )

    def din(name, shape, dt=F32):
        return nc.dram_tensor(name, list(shape), dt, kind="ExternalInput").ap()

    xT = din("xT", (D, S))                    # per-core batch
    xq_in = din("xq", (D, SQ))                # per-core q column slice
    biasT = din("biasT", (S, SQ), BF16)       # per-core, x0.2 prescaled
    ropeq = din("ropeq", (SQ, (3 if has_qb else 2) * 4 * P), BF16)  # per-core
    ropek = din("ropek", (S, (3 if has_kb else 2) * P), BF16)
    wkv = din("wkv", (D, KV))
    wq = din("wq", (D, H * DQ))
    wo = din("wo", (H * DV, D), BF16)
    consts = din("consts", (P, CW))
    out = nc.dram_tensor("out", [SQ, D], F32, kind="ExternalOutput").ap()

    TT = mybir.AluOpType
    AF = mybir.ActivationFunctionType
    AX = mybir.AxisListType

    qjc0 = None  # chunks covered by this core's q slice: set via partition id?
    # Each core's q slice differs, but the PROGRAM is shared across cores.
    # xq is just a slice of xT columns; we DMA those 4 chunks twice (once into
    # the stream tile for kv, once into xq).  Cheap (3.1MB extra DMA).

    with tile.TileContext(nc) as tc, ExitStack() as ctx:
        const = ctx.enter_context(tc.tile_pool(name="const", bufs=1))
        persist = ctx.enter_context(tc.tile_pool(name="persist", bufs=1))

        cst = const.tile([P, CW], F32)
        nc.sync.dma_start(cst[:], consts)
        ident = cst[:, C_ID : C_ID + P]
        ident_bf = const.tile([P, P], BF16)
        nc.vector.tensor_copy(ident_bf[:], ident)
        eps_ap = cst[:, C_EPS : C_EPS + 1]

        kT_sb = persist.tile([P, JC, P], F32)         # [dk, jc, key]
        vrow_sb = persist.tile([P, JC, VP], BF16)     # [key, jc, v|1|pad]
        qT_sb = persist.tile([P, H, SQ], F32)         # [dq, h, q]

        nc.vector.memset(vrow_sb[:, :, DV : DV + 1], 1.0)
        nc.vector.memset(vrow_sb[:, :, DV + 1 :], 0.0)

        qres_cm = tc.tile_pool(name="qres", bufs=1)
        qres = qres_cm.__enter__()
        xq_sb = qres.tile([P, DC, SQ], F32)

        # =====================================================
        # Phase KV
        # =====================================================
        with (
            tc.tile_pool(name="kvs", bufs=1) as kvs,
            tc.tile_pool(name="kvw", bufs=3) as kvw,
            tc.tile_pool(name="xsp", bufs=2) as xsp,
            tc.tile_pool(name="kn4p", bufs=3) as kn4p,
            tc.tile_pool(name="kvps", bufs=3, space="PSUM") as kvps,
            tc.tile_pool(name="kvtp", bufs=2, space="PSUM") as kvtp,
        ):
            wkv_sb = kvs.tile([P, DC, KV], F32)
            nc.sync.dma_start(
                _r(wkv_sb[:]), _r(wkv.rearrange("(c p) f -> p c f", p=P))
            )
            ropek_sb = kvs.tile([P, JC, (3 if has_kb else 2) * P], BF16)
            nc.sync.dma_start(
                ropek_sb[:], ropek.rearrange("(j p) f -> p j f", p=P)
            )
            x3 = xT.rearrange("(c p) s -> p c s", p=P)

            invn = cst[:, C_INV : C_INV + 2]

            kn4 = None
            xslab = None
            for jc in range(JC):
                if jc % 4 == 0:
                    kn4 = kn4p.tile([P, 4, P], F32, tag="kn4")
                if jc % 2 == 0:
                    xslab = xsp.tile([P, DC, 2 * P], F32, tag="xslab")
                    nc.sync.dma_start(
                        _r(xslab[:]),
                        _r(x3[:, :, jc * P : (jc + 2) * P]),
                    )
                xc = xslab[:, :, (jc % 2) * P : (jc % 2 + 1) * P]
                kv_ps = kvps.tile([P, KV], F32, tag="kv_ps")
                for dc in range(DC):
                    nc.tensor.matmul(
                        kv_ps[:],
                        _r(xc[:, dc, :]),
                        _r(wkv_sb[:, dc, :]),
                        start=(dc == 0),
                        stop=(dc == DC - 1),
                    )
                # evict + folded rms1 bias; accum gives the LN sums free
                kvr = kvw.tile([P, KV], F32, tag="kvr")
                st = kvw.tile([P, 16], F32, tag="st")
                nc.vector.scalar_tensor_tensor(
                    kvr[:, :DK], kv_ps[:, :DK], 1.0,
                    cst[:, C_BKV : C_BKV + DK], TT.mult, TT.add,
                    accum_out=st[:, 0:1],
                )
                nc.vector.scalar_tensor_tensor(
                    kvr[:, DK:], kv_ps[:, DK:], 1.0,
                    cst[:, C_BKV + DK : C_BKV + KV], TT.mult, TT.add,
                    accum_out=st[:, 1:2],
                )
                # sumsq on ACT (square with accumulator)
                sq = kvw.tile([P, KV], F32, tag="sq")
                nc.scalar.activation(sq[:, :DK], kvr[:, :DK], AF.Square,
                                     accum_out=st[:, 2:3])
                nc.scalar.activation(sq[:, DK:], kvr[:, DK:], AF.Square,
                                     accum_out=st[:, 3:4])
                # smu = s1*invn ; var = (s2 - smu*s1)*invn ; rs = rsqrt(var+eps)
                nc.vector.tensor_tensor(st[:, 4:6], st[:, 0:2], invn, TT.mult)
                nc.vector.tensor_tensor(st[:, 6:8], st[:, 4:6], st[:, 0:2],
                                        TT.mult)
                nc.vector.tensor_tensor(st[:, 8:10], st[:, 2:4], st[:, 6:8],
                                        TT.subtract)
                nc.vector.tensor_tensor(st[:, 8:10], st[:, 8:10], invn,
                                        TT.mult)
                nc.scalar.activation(st[:, 10:12], st[:, 8:10], AF.Sqrt,
                                     bias=eps_ap)
                nc.vector.reciprocal(st[:, 10:12], st[:, 10:12])
                # nmr = -smu*rs  (bias for the ACT-side LN apply)
                nc.vector.scalar_tensor_tensor(
                    st[:, 12:14], st[:, 4:6], -1.0, st[:, 10:12],
                    TT.mult, TT.mult,
                )
                # apply (ACT): out = in*rs + (-smu*rs)
                nc.scalar.activation(
                    kn4[:, jc % 4, :], kvr[:, :DK], AF.Identity,
                    bias=st[:, 12:13], scale=st[:, 10:11],
                )
                nc.scalar.activation(
                    vrow_sb[:, jc, :DV], kvr[:, DK:], AF.Identity,
                    bias=st[:, 13:14], scale=st[:, 11:12],
                )
                if jc % 4 == 3:
                    j0 = jc - 3
                    ck = ropek_sb[:, j0 : j0 + 4, 0:P]
                    sk = ropek_sb[:, j0 : j0 + 4, P : 2 * P]
                    r1 = kvw.tile([P, 4, P], F32, tag="r1")
                    r2 = kvw.tile([P, 4, P], F32, tag="r2")
                    nc.vector.tensor_tensor(r1[:], kn4[:], ck, TT.mult)
                    nc.gpsimd.tensor_tensor(
                        r2[:, :, 0:HALF], kn4[:, :, HALF:P],
                        sk[:, :, 0:HALF], TT.mult,
                    )
                    nc.gpsimd.tensor_tensor(
                        r2[:, :, HALF:P], kn4[:, :, 0:HALF],
                        sk[:, :, HALF:P], TT.mult,
                    )
                    kr = kvw.tile([P, 4, P], F32, tag="kr")
                    nc.vector.tensor_tensor(kr[:], r1[:], r2[:], TT.add)
                    if has_kb:
                        bk = ropek_sb[:, j0 : j0 + 4, 2 * P : 3 * P]
                        nc.vector.tensor_tensor(kr[:], kr[:], bk, TT.add)
                    for t in range(4):
                        scr = kvtp.tile([P, P], F32, tag="scr")
                        nc.tensor.transpose(scr[:], kr[:, t, :], ident)
                        nc.vector.tensor_copy(_r(kT_sb[:, j0 + t, :]), scr[:])

            # prefetch q weights + tables mid-phase (SBUF timing)
            wq_sb = qres.tile([P, DC, H * DQ], F32)
            nc.sync.dma_start(
                _r(wq_sb[:]), _r(wq.rearrange("(c p) f -> p c f", p=P))
            )
            ropeq_sb = qres.tile([P, SC, (3 if has_qb else 2) * 4 * P], BF16)
            nc.sync.dma_start(
                ropeq_sb[:], ropeq.rearrange("(s p) f -> p s f", p=P)
            )

        # q slice of x (this core's own rows), one transfer
        nc.sync.dma_start(
            _r(xq_sb[:]), _r(xq_in.rearrange("(c p) s -> p c s", p=P))
        )

        # =====================================================
        # Phase Q  (two groups of 4 heads)
        # =====================================================
        with (
            tc.tile_pool(name="qw", bufs=3) as qw,
            tc.tile_pool(name="qps", bufs=3, space="PSUM") as qps,
            tc.tile_pool(name="qtp", bufs=2, space="PSUM") as qtp,
        ):
            for g in range(2):
                f0 = g * HG * DQ
                for ic in range(SC):
                    q_ps = qps.tile([P, HG * DQ], F32, tag="q_ps")
                    for dc in range(DC):
                        nc.tensor.matmul(
                            q_ps[:],
                            _r(xq_sb[:, dc, ic * P : (ic + 1) * P]),
                            _r(wq_sb[:, dc, f0 : f0 + HG * DQ]),
                            start=(dc == 0),
                            stop=(dc == DC - 1),
                        )
                    qr = qw.tile([P, HG * DQ], F32, tag="qr")
                    nc.vector.tensor_tensor(
                        qr[:], q_ps[:],
                        cst[:, C_BQ + f0 : C_BQ + f0 + HG * DQ], TT.add
                    )
                    sqq = qw.tile([P, HG * DQ], F32, tag="sqq")
                    stq = qw.tile([P, 28], F32, tag="stq")
                    qr3 = qr[:].rearrange("p (h f) -> p h f", h=HG)
                    nc.vector.tensor_reduce(stq[:, 0:4], qr3, AX.X, TT.add)
                    for t in range(HG):
                        nc.scalar.activation(
                            sqq[:, t * DQ : (t + 1) * DQ],
                            qr[:, t * DQ : (t + 1) * DQ],
                            AF.Square, accum_out=stq[:, 4 + t : 5 + t],
                        )
                    nc.vector.tensor_scalar(
                        stq[:, 8:12], stq[:, 0:4], 1.0 / DQ, None, TT.mult
                    )
                    nc.vector.tensor_tensor(
                        stq[:, 12:16], stq[:, 8:12], stq[:, 0:4], TT.mult
                    )
                    nc.vector.tensor_tensor(
                        stq[:, 16:20], stq[:, 4:8], stq[:, 12:16], TT.subtract
                    )
                    nc.vector.tensor_scalar(
                        stq[:, 16:20], stq[:, 16:20], 1.0 / DQ, None, TT.mult
                    )
                    nc.scalar.activation(
                        stq[:, 20:24], stq[:, 16:20], AF.Sqrt, bias=eps_ap
                    )
                    nc.vector.reciprocal(stq[:, 20:24], stq[:, 20:24])
                    nc.vector.scalar_tensor_tensor(
                        stq[:, 24:28], stq[:, 8:12], -1.0, stq[:, 20:24],
                        TT.mult, TT.mult,
                    )
                    qn4 = qw.tile([P, HG, DQ], F32, tag="qn4")
                    for t in range(HG):
                        nc.scalar.activation(
                            qn4[:, t, :], qr[:, t * DQ : (t + 1) * DQ],
                            AF.Identity,
                            bias=stq[:, 24 + t : 25 + t],
                            scale=stq[:, 20 + t : 21 + t],
                        )
                    cq = ropeq_sb[:, ic, 0 : HG * P].rearrange(
                        "p (h f) -> p h f", h=HG)
                    sqt = ropeq_sb[:, ic, HG * P : 2 * HG * P].rearrange(
                        "p (h f) -> p h f", h=HG)
                    r1 = qw.tile([P, HG, DQ], F32, tag="qr1")
                    r2 = qw.tile([P, HG, DQ], F32, tag="qr2")
                    nc.vector.tensor_tensor(r1[:], qn4[:], cq, TT.mult)
                    nc.gpsimd.tensor_tensor(
                        r2[:, :, 0:HALF], qn4[:, :, HALF:DQ],
                        sqt[:, :, 0:HALF], TT.mult,
                    )
                    nc.gpsimd.tensor_tensor(
                        r2[:, :, HALF:DQ], qn4[:, :, 0:HALF],
                        sqt[:, :, HALF:DQ], TT.mult,
                    )
                    qrope = qw.tile([P, HG, DQ], F32, tag="qrope")
                    nc.vector.tensor_tensor(qrope[:], r1[:], r2[:], TT.add)
                    if has_qb:
                        bq4 = ropeq_sb[:, ic, 2 * HG * P : 3 * HG * P
                                       ].rearrange("p (h f) -> p h f", h=HG)
                        nc.vector.tensor_tensor(qrope[:], qrope[:], bq4,
                                                TT.add)
                    for t in range(HG):
                        scr = qtp.tile([P, P], F32, tag="qscr")
                        nc.tensor.transpose(scr[:], qrope[:, t, :], ident)
                        nc.vector.tensor_copy(
                            _r(qT_sb[:, g * HG + t, ic * P : (ic + 1) * P]),
                            scr[:],
                        )

        qres_cm.__exit__(None, None, None)

        # =====================================================
        # Attention
        # =====================================================
        with tc.tile_pool(name="wop", bufs=1) as wop:
            yatt_sb = wop.tile([P, SC, H * DV], BF16)  # [q, sc, hdv]
            biasT_sb = wop.tile([P, JC, SQ], BF16)
            nc.sync.dma_start(
                biasT_sb[:], biasT.rearrange("(j p) i -> p j i", p=P)
            )
            wo_sb = wop.tile([P, DC, D], BF16)
            nc.sync.dma_start(
                wo_sb[:], wo.rearrange("(c p) f -> p c f", p=P)
            )

            with (
                tc.tile_pool(name="att", bufs=2) as att,
                tc.tile_pool(name="apsum", bufs=2, space="PSUM") as aps,
                tc.tile_pool(name="ypsum", bufs=1, space="PSUM") as yps,
            ):
                y_ps = [
                    yps.tile([P, VP], F32, tag=f"y{ic}", name=f"y{ic}")
                    for ic in range(SC)
                ]
                for h in range(H):
                    for jp in range(JC // 4):      # pairs of 2-chunk groups
                        zt4 = att.tile([P, 4, SQ], BF16, tag="zt4")
                        for half in range(2):
                            jg = jp * 2 + half
                            pq = aps.tile([P, 2, SQ], F32, tag="pq")
                            for c in range(2):
                                jc = jg * 2 + c
                                nc.tensor.matmul(
                                    pq[:, c, :],
                                    _r(kT_sb[:, jc, :]),
                                    _r(qT_sb[:, h, :]),
                                    start=True, stop=True,
                                )
                            nc.vector.scalar_tensor_tensor(
                                zt4[:, half * 2 : half * 2 + 2, :],
                                pq[:],
                                1.0 / SOFTCAP,
                                biasT_sb[:, jg * 2 : jg * 2 + 2, :],
                                TT.mult, TT.add,
                            )
                        tt4 = att.tile([P, 4, SQ], BF16, tag="tt4")
                        nc.scalar.activation(tt4[:], zt4[:], AF.Tanh)
                        pt4 = att.tile([P, 4, SQ], BF16, tag="pt4")
                        nc.scalar.activation(pt4[:], tt4[:], AF.Exp,
                                             scale=SOFTCAP)
                        for c in range(4):
                            jc = jp * 4 + c
                            for ic in range(SC):
                                nc.tensor.matmul(
                                    y_ps[ic][:],
                                    pt4[:, c, ic * P : (ic + 1) * P],
                                    vrow_sb[:, jc, :],
                                    start=(jc == 0),
                                    stop=(jc == JC - 1),
                                )
                    for ic in range(SC):
                        recip = att.tile([P, 1], F32, tag="recip")
                        nc.vector.reciprocal(
                            recip[:], y_ps[ic][:, DV : DV + 1]
                        )
                        nc.vector.tensor_scalar(
                            yatt_sb[:, ic, h * DV : (h + 1) * DV],
                            y_ps[ic][:, :DV],
                            recip[:, 0:1], None, TT.mult,
                        )

            # =================================================
            # Output projection
            # =================================================
            with (
                tc.tile_pool(name="op", bufs=2) as op,
                tc.tile_pool(name="opsum", bufs=1, space="PSUM") as ops,
                tc.tile_pool(name="otps", bufs=2, space="PSUM") as otps,
            ):
                for sc in range(SC):
                    yT = op.tile([P, DC, P], BF16, tag="yT")
                    for fc in range(DC):
                        pt2 = otps.tile([P, P], BF16, tag="yt")
                        nc.tensor.transpose(
                            pt2[:],
                            yatt_sb[:, sc, fc * P : (fc + 1) * P],
                            ident_bf[:],
                        )
                        nc.vector.tensor_copy(yT[:, fc, :], pt2[:])
                    o_ps = [
                        ops.tile([P, 512], F32, tag=f"o{n}", name=f"o{n}_{sc}")
                        for n in range(3)
                    ]
                    for fc in range(DC):
                        for n in range(3):
                            nc.tensor.matmul(
                                o_ps[n][:],
                                yT[:, fc, :],
                                wo_sb[:, fc, n * 512 : (n + 1) * 512],
                                start=(fc == 0),
                                stop=(fc == DC - 1),
                            )
                    o_sb = op.tile([P, D], F32, tag="o_sb")
                    for n in range(3):
                        nc.vector.tensor_tensor(
                            o_sb[:, n * 512 : (n + 1) * 512],
                            o_ps[n][:],
                            cst[:, C_BO + n * 512 : C_BO + (n + 1) * 512],
                            TT.add,
                        )
                    nc.sync.dma_start(out[sc * P : (sc + 1) * P, :], o_sb[:])

    nc.compile()
    return nc


def _rope_tables(n, g, b, scale, start=0):
    """Full-width tables (f32): out = xhat*C + xswap*Sw (+ B)."""
    f32 = np.float32
    freqs = 1.0 / (ROPE_BASE ** (np.arange(HALF, dtype=f32) / HALF))
    ang = (start + np.arange(n, dtype=f32))[:, None] * freqs[None, :]
    cos, sin = np.cos(ang).astype(f32), np.sin(ang).astype(f32)
    g1, g2 = g[:HALF], g[HALF:]
    b1, b2 = b[:HALF], b[HALF:]
    C = np.concatenate([g1 * cos, g2 * cos], axis=1) * scale
    Sw = np.concatenate([-g2 * sin, g1 * sin], axis=1) * scale
    Bt = np.concatenate([b1 * cos - b2 * sin, b1 * sin + b2 * cos],
                        axis=1) * scale
    return C.astype(f32), Sw.astype(f32), Bt.astype(f32)


def _host_prep(inputs):
    f32 = np.float32
    x = np.asarray(inputs["x"], f32)
    bias = np.asarray(inputs["attention_bias"], f32)
    g1 = np.asarray(inputs["g1"], f32)
    b1 = np.asarray(inputs["b1"], f32)
    rr1 = np.asarray(inputs["rrms1"], f32)
    Wq = np.asarray(inputs["Wq"], f32)
    Wk = np.asarray(inputs["Wk"], f32)
    Wv = np.asarray(inputs["Wv"], f32)
    qg = np.asarray(inputs["qg"], f32)
    qb = np.asarray(inputs["qb"], f32)
    kg = np.asarray(inputs["kg"], f32)
    kb = np.asarray(inputs["kb"], f32)
    vg = np.asarray(inputs["vg"], f32)
    vb = np.asarray(inputs["vb"], f32)
    Wo = np.asarray(inputs["Wo"], f32)
    bo = np.asarray(inputs["bo"], f32)
    g2 = np.asarray(inputs["g2"], f32)
    b2 = np.asarray(inputs["b2"], f32)
    rr2 = np.asarray(inputs["rrms2"], f32)

    has_kb = bool(np.any(kb != 0))
    has_qb = bool(np.any(qb != 0))

    scale1 = (g1 * (1.0 / np.sqrt(rr1 + EPS_RMS))).astype(f32)
    Wkv = np.concatenate([Wk * scale1[:, None], Wv * scale1[:, None]],
                         axis=1).astype(f32)
    bkv = np.concatenate([b1 @ Wk, b1 @ Wv]).astype(f32)
    Wq_e = (Wq * scale1[:, None]).astype(f32)
    bq_row = (b1 @ Wq).astype(f32)

    sc_q = f32(DQ) ** f32(-0.5)
    scale2 = (g2 * (1.0 / np.sqrt(rr2 + EPS_RMS))).astype(f32)
    vg_t = np.tile(vg, H).astype(f32)
    vb_t = np.tile(vb, H).astype(f32)
    Wo_f = (vg_t[:, None] * Wo * scale2[None, :]).astype(f32)
    bo_f = ((vb_t @ Wo + bo) * scale2 + b2).astype(f32)

    # k rope tables (full S)
    Ck, Sk, Bk = _rope_tables(S, kg, kb, f32(1.0))
    ropek_cols = [Ck, Sk] + ([Bk] if has_kb else [])
    ropek = np.concatenate(ropek_cols, axis=1).astype(NPBF16)

    rep = lambda v: np.ascontiguousarray(
        np.broadcast_to(v[None, :], (P, v.shape[0])), dtype=f32)
    consts = np.zeros((P, CW), f32)
    consts[:, C_ID : C_ID + P] = np.eye(P, dtype=f32)
    consts[:, C_BKV : C_BKV + KV] = rep(bkv)
    consts[:, C_BQ : C_BQ + H * DQ] = rep(bq_row)
    consts[:, C_BO : C_BO + D] = rep(bo_f)
    consts[:, C_INV] = f32(1.0 / DK)
    consts[:, C_INV + 1] = f32(1.0 / DV)
    consts[:, C_EPS] = f32(EPS_LN)

    shared = {
        "wkv": np.ascontiguousarray(Wkv),
        "wq": np.ascontiguousarray(Wq_e),
        "wo": np.ascontiguousarray(Wo_f.astype(NPBF16)),
        "ropek": np.ascontiguousarray(ropek),
        "consts": np.ascontiguousarray(consts),
    }

    xTs = [np.ascontiguousarray(x[b].T) for b in range(B)]
    in_maps = []
    for c in range(NCORES):
        b = c // 4
        s0 = (c % 4) * SQ
        m = dict(shared)
        m["xT"] = xTs[b]
        m["xq"] = np.ascontiguousarray(xTs[b][:, s0 : s0 + SQ])
        m["biasT"] = np.ascontiguousarray(
            (bias[0, 0, s0 : s0 + SQ, :].T * (1.0 / SOFTCAP)).astype(NPBF16)
        )
        Cq, Sq, Bq = _rope_tables(SQ, qg, qb, sc_q, start=s0)
        rq_cols = [np.tile(Cq, (1, HG)), np.tile(Sq, (1, HG))]
        if has_qb:
            rq_cols.append(np.tile(Bq, (1, HG)))
        m["ropeq"] = np.ascontiguousarray(
            np.concatenate(rq_cols, axis=1).astype(NPBF16)
        )
        in_maps.append(m)
    return in_maps, (has_kb, has_qb)


_NC_CACHE = {}


def _get_nc(flags=(False, False)):
    if flags not in _NC_CACHE:
        _NC_CACHE[flags] = build_program(*flags)
    return _NC_CACHE[flags]


def kernel(**inputs) -> np.ndarray:
    in_maps, flags = _host_prep(inputs)
    nc = _get_nc(flags)
    res = bass_utils.run_bass_kernel_spmd(
        nc, in_maps, core_ids=list(range(NCORES))
    )
    outs = res.results
    full = np.empty((B, S, D), np.float32)
    for c in range(NCORES):
        b = c // 4
        s0 = (c % 4) * SQ
        full[b, s0 : s0 + SQ, :] = outs[c]["out"]
    return full


if __name__ == "__main__":
    nc = _get_nc()
    print("build + compile OK")
